# revision 15
# baseline (speedup 1.0000x reference)
"""Trainium2 Bass kernel for the AutoregressiveLSTM problem.

Data-parallel over 8 NeuronCores: batch 2048 -> 256 per core.

Per-core layout ("feature-major"): the LSTM state h and cell c live as
hT [H, B] packed into SBUF tiles [128, 2*128] (H-chunk-major), so the
recurrent matmul gatesT[4H, B] = w_hh @ h needs no per-step transposes
and the FC layer can use hT chunks directly as the stationary operand.

The input-side gate contribution xg[t] = W2[x[:, t]] (W2 = emb_table @
w_ih.T + b_ih + b_hh, folded on host) is accumulated into the same PSUM
banks via one-hot matmuls with K=36.

exp/ln (log-softmax) are deferred to a single end phase because Sigmoid/
Tanh/Relu share one ACT table set while Exp/Ln live in another (~2.7us
per table switch if interleaved).

The emb output is a pure gather -> indirect DMA, no compute engines.
"""

import math
import os
from contextlib import ExitStack

import numpy as np

import concourse.bass as bass
import concourse.tile as tile
from concourse import bacc, mybir
from concourse.bass import AP

FP32 = mybir.dt.float32
I32 = mybir.dt.int32
AF = mybir.ActivationFunctionType
ALU = mybir.AluOpType

# Problem constants (hardcoded per contract)
B_FULL, T, V, E, H = 2048, 128, 36, 128, 256
NCORES = 8
B = B_FULL // NCORES          # 256 per core
NB = 2                        # batch chains per core
BC = B // NB                  # 128 per chain
G4 = 4 * H                    # 1024
NSTEP = T - 1                 # 127 LSTM steps
CH = 7                        # steps per chunk (14 FC slots of 36 = 504 <= 512 psum bank)
NTOK = B * T                  # 32768 tokens per core (for emb gather)
IGNORE = 35

# gatesT PSUM col-block m covers gate rows R[m] (PyTorch gate order i,f,g,o
# in the 4H dim). Block packing: bank0 = [i0 i1 f0 f1], bank1 = [o0 o1 g0 g1]
# so sigmoid covers cols 0:768 and tanh covers cols 768:1024.
BLOCK_ROWS = [
    (0, 128), (128, 256),          # i
    (256, 384), (384, 512),        # f
    (768, 896), (896, 1024),       # o
    (512, 640), (640, 768),        # g
]

EMB_GCHUNK = 1024              # tokens per dma_gather call (desc-ring limit)
N_EMB_G = NTOK // EMB_GCHUNK   # 32
EMB_ROWS = EMB_GCHUNK // 128   # 8 gathered rows per out free slot


def _bc(ap: AP, dims) -> AP:
    """Build a broadcast/strided view of an AP: dims = [(step, count), ...]
    appended after the partition dim. step refers to the flat free offset of
    the underlying tile."""
    part = ap.ap[0]
    return AP(ap.tensor, ap.offset, [list(part)] + [[s, c] for s, c in dims])


def build_program(num_devices: int = NCORES, nstep: int = NSTEP):
    nc = bacc.Bacc(
        "TRN2",
        target_bir_lowering=False,
        debug=False,
        enable_asserts=False,
        num_devices=num_devices,
    )

    # ---- DRAM tensors ----
    xidx16 = nc.dram_tensor("xidx16", [128, NTOK // 16], mybir.dt.int16,
                            kind="ExternalInput").ap()
    xflat32 = nc.dram_tensor("xflat32", [NTOK], FP32, kind="ExternalInput").ap()
    xT = nc.dram_tensor("xT", [T, B], FP32, kind="ExternalInput").ap()
    whhT_d = nc.dram_tensor("whhT", [H, G4], FP32, kind="ExternalInput").ap()
    W2_d = nc.dram_tensor("W2", [V, G4], FP32, kind="ExternalInput").ap()
    wfcT_d = nc.dram_tensor("wfcT", [H, V], FP32, kind="ExternalInput").ap()
    bfc_d = nc.dram_tensor("bfc", [V], FP32, kind="ExternalInput").ap()
    etab_d = nc.dram_tensor("etab", [V, E], FP32, kind="ExternalInput").ap()

    emb_out = nc.dram_tensor("emb_out", [NTOK, E], FP32, kind="ExternalOutput").ap()
    lp_out = nc.dram_tensor("lp_out", [B, nstep, V], FP32, kind="ExternalOutput").ap()
    nll_out = nc.dram_tensor("nll_out", [128, NB], FP32, kind="ExternalOutput").ap()

    with tile.TileContext(nc) as tc, ExitStack() as ctx:
        const_p = ctx.enter_context(tc.tile_pool(name="const", bufs=1))
        ring_p = ctx.enter_context(tc.tile_pool(name="ring", bufs=1))
        state_p = ctx.enter_context(tc.tile_pool(name="state", bufs=2))
        work_p = ctx.enter_context(tc.tile_pool(name="work", bufs=2))
        oh_p = ctx.enter_context(tc.tile_pool(name="oh", bufs=2))
        emb_p = ctx.enter_context(tc.tile_pool(name="emb", bufs=2))
        end_p = ctx.enter_context(tc.tile_pool(name="end", bufs=2))
        gates_pp = ctx.enter_context(tc.tile_pool(name="gates", bufs=1, space="PSUM"))
        fc_pp = ctx.enter_context(tc.tile_pool(name="fc", bufs=2, space="PSUM"))

        # ---- static loads ----
        whh_sb = []
        for j in range(2):
            wt = const_p.tile([128, G4], FP32, tag=f"whh{j}")
            nc.sync.dma_start(wt[:, :], whhT_d[j * 128:(j + 1) * 128, :])
            whh_sb.append(wt)
        W2_sb = const_p.tile([V, G4], FP32, tag="w2")
        nc.sync.dma_start(W2_sb[:, :], W2_d[:, :])
        wfc_sb = []
        for j in range(2):
            wt = const_p.tile([128, V], FP32, tag=f"wfc{j}")
            nc.sync.dma_start(wt[:, :], wfcT_d[j * 128:(j + 1) * 128, :])
            wfc_sb.append(wt)
        etab_sb = const_p.tile([V, E], FP32, tag="etab")
        nc.sync.dma_start(etab_sb[:, :], etab_d[:, :])

        bfc_tile = const_p.tile([128, 2 * CH * V], FP32, tag="bfc")
        nc.sync.dma_start(
            bfc_tile[:, :],
            AP(bfc_d.tensor, 0, [[0, 128], [0, 2 * CH], [1, V]]),
        )

        iota36 = const_p.tile([V, 1], FP32, tag="iota36")
        nc.gpsimd.iota(iota36[:, :], pattern=[[0, 1]], base=0,
                       channel_multiplier=1,
                       allow_small_or_imprecise_dtypes=True)
        iota_row = const_p.tile([128, V], FP32, tag="iotar")
        nc.gpsimd.iota(iota_row[:, :], pattern=[[1, V]], base=0,
                       channel_multiplier=0,
                       allow_small_or_imprecise_dtypes=True)

        # x in batch-chain layout: xB[p, q, t] = x[q*128+p, t]
        xB = const_p.tile([128, NB, T], FP32, tag="xB")
        nc.sync.dma_start(
            xB[:, :, :],
            AP(xflat32.tensor, 0, [[T, 128], [128 * T, NB], [1, T]]),
        )

        # rings kept for the whole scan
        relu_ring = ring_p.tile([128, nstep * NB * V], FP32, tag="relu_ring")
        sel_ring = ring_p.tile([128, nstep * NB], FP32, tag="sel_ring")

        # ---- emb output via dma_gather (independent of the scan) ----
        idx16_sb = const_p.tile([128, NTOK // 16], mybir.dt.int16, tag="idx16")
        nc.sync.dma_start(idx16_sb[:, :], xidx16[:, :])
        npc = EMB_GCHUNK // 16     # idx columns per gather chunk
        for gi in range(N_EMB_G):
            gbuf = emb_p.tile([128, EMB_ROWS, E], FP32, tag="gbuf")
            nc.gpsimd.dma_gather(
                out_ap=gbuf[:, :, :],
                in_ap=etab_d[:, :],
                idxs_ap=idx16_sb[:, gi * npc:(gi + 1) * npc],
                num_idxs=EMB_GCHUNK,
                num_idxs_reg=EMB_GCHUNK,
                elem_size=E,
            )
            nc.sync.dma_start(
                AP(emb_out.tensor, gi * EMB_GCHUNK * E,
                   [[E, 128], [128 * E, EMB_ROWS], [1, E]]),
                gbuf[:, :, :],
            )

        # ---- the scan ----
        h_prev = [None, None]   # per chain: SBUF tile [128, 256] (hT chunks)
        c_prev = [None, None]   # per chain: SBUF tile [128, 256]

        n_chunks = math.ceil(nstep / CH)
        for ci in range(n_chunks):
            t0 = ci * CH
            t1 = min(t0 + CH, nstep)
            nch = t1 - t0
            nslot = nch * NB

            # one-hot (vocab-major) for xg: ohT[v, (t_rel, b)] = (x[b, t]==v)
            xbc = oh_p.tile([V, nch * B], FP32, tag="xbc")
            nc.sync.dma_start(
                xbc[:, :],
                AP(xT.tensor, t0 * B, [[0, V], [B, nch], [1, B]]),
            )
            ohT = oh_p.tile([V, nch * B], FP32, tag="ohT")
            nc.gpsimd.tensor_scalar(
                ohT[:, :], xbc[:, :], iota36[:, :1], None, ALU.is_equal)

            # one-hot (token-major) of targets for the NLL selection:
            # ohN[p, (t_rel, q, v)] = (x[q*128+p, t+1] == v)
            ohN = oh_p.tile([128, nslot * V], FP32, tag="ohN")
            xb_ap = xB[:, :, :]
            nc.vector.tensor_tensor(
                ohN[:, :],
                AP(xb_ap.tensor, xb_ap.offset + t0 + 1,
                   [list(xb_ap.ap[0]), [1, nch], [T, NB], [0, V]]),
                _bc(iota_row[:, :], [(0, nch), (0, NB), (1, V)]),
                ALU.is_equal,
            )

            fc_ps = fc_pp.tile([128, nslot * V], FP32, tag="fcps")

            for tr in range(nch):
                t = t0 + tr
                for q in range(NB):
                    gates = gates_pp.tile([128, G4], FP32, tag=f"gates{q}")
                    # input-side one-hot matmuls (K=36), first MM per bank
                    # clears it (start=True)
                    for m in range(8):
                        r0, r1 = BLOCK_ROWS[m]
                        nc.tensor.matmul(
                            gates[:, m * 128:(m + 1) * 128],
                            W2_sb[:, r0:r1],
                            ohT[:, tr * B + q * BC: tr * B + q * BC + BC],
                            start=(m % 4 == 0),
                            stop=(t == 0 and m % 4 == 3),
                        )
                    # recurrent matmuls
                    if t > 0:
                        for m in range(8):
                            r0, r1 = BLOCK_ROWS[m]
                            for j in range(2):
                                nc.tensor.matmul(
                                    gates[:, m * 128:(m + 1) * 128],
                                    whh_sb[j][:, r0:r1],
                                    h_prev[q][:, j * 128:(j + 1) * 128],
                                    start=False,
                                    stop=(m % 4 == 3 and j == 1),
                                )

                    ifo = work_p.tile([128, 768], FP32, tag=f"ifo{q}")
                    nc.scalar.activation(ifo[:, :], gates[:, 0:768], AF.Sigmoid)
                    gg = work_p.tile([128, 256], FP32, tag=f"g{q}")
                    nc.scalar.activation(gg[:, :], gates[:, 768:1024], AF.Tanh)

                    ig = work_p.tile([128, 256], FP32, tag=f"ig{q}")
                    nc.vector.tensor_tensor(
                        ig[:, :], ifo[:, 0:256], gg[:, :], ALU.mult)

                    if t == 0:
                        c_new = ig
                    else:
                        fcs = work_p.tile([128, 256], FP32, tag=f"fc{q}")
                        nc.gpsimd.tensor_tensor(
                            fcs[:, :], ifo[:, 256:512], c_prev[q][:, :], ALU.mult)
                        c_new = state_p.tile([128, 256], FP32, tag=f"c{q}")
                        nc.vector.tensor_tensor(
                            c_new[:, :], ig[:, :], fcs[:, :], ALU.add)
                    c_prev[q] = c_new

                    th = work_p.tile([128, 256], FP32, tag=f"th{q}")
                    nc.scalar.activation(th[:, :], c_new[:, :], AF.Tanh)
                    h_new = state_p.tile([128, 256], FP32, tag=f"h{q}")
                    nc.vector.tensor_tensor(
                        h_new[:, :], ifo[:, 512:768], th[:, :], ALU.mult)
                    h_prev[q] = h_new

                    # FC: logits[tokens, V] for this (t, q) into its fc slot
                    slot = tr * NB + q
                    for j in range(2):
                        nc.tensor.matmul(
                            fc_ps[:, slot * V:(slot + 1) * V],
                            h_new[:, j * 128:(j + 1) * 128],
                            wfc_sb[j][:, :],
                            start=(slot == 0 and j == 0),
                            stop=(slot == nslot - 1 and j == 1),
                        )

            # chunk epilogue: bias, relu into the ring, NLL selection
            biased = work_p.tile([128, nslot * V], FP32, tag="biased")
            nc.vector.tensor_tensor(
                biased[:, :], fc_ps[:, :], bfc_tile[:, :nslot * V], ALU.add)
            nc.scalar.activation(
                relu_ring[:, t0 * NB * V:(t0 * NB + nslot) * V],
                biased[:, :], AF.Relu)
            selp = work_p.tile([128, nslot * V], FP32, tag="selp")
            rr_ap = relu_ring[:, :]
            nc.gpsimd.tensor_tensor(
                selp[:, :],
                AP(rr_ap.tensor, rr_ap.offset + t0 * NB * V,
                   [list(rr_ap.ap[0]), [1, nslot * V]]),
                ohN[:, :], ALU.mult)
            nc.vector.tensor_reduce(
                out=sel_ring[:, t0 * NB:t0 * NB + nslot],
                in_=AP(selp[:, :].tensor, selp[:, :].offset,
                       [list(selp[:, :].ap[0]), [V, nslot], [1, V]]),
                axis=mybir.AxisListType.X,
                op=ALU.add,
            )

        # ---- end phase: exp/ln (one ACT table switch), log-probs, NLL ----
        logZ = ring_p.tile([128, nstep * NB], FP32, tag="logZ")
        n_slots_total = nstep * NB
        piece_slots = 64
        off = 0
        while off < n_slots_total:
            ps = min(piece_slots, n_slots_total - off)
            expx = end_p.tile([128, piece_slots * V], FP32, tag="expx")
            nc.scalar.activation(
                expx[:, :ps * V],
                relu_ring[:, off * V:(off + ps) * V], AF.Exp)
            nc.vector.tensor_reduce(
                out=logZ[:, off:off + ps],
                in_=AP(expx[:, :].tensor, expx[:, :].offset,
                       [list(expx[:, :].ap[0]), [V, ps], [1, V]]),
                axis=mybir.AxisListType.X,
                op=ALU.add,
            )
            off += ps
        sumsZ = logZ
        logZ = ring_p.tile([128, nstep * NB], FP32, tag="logZ2")
        nc.scalar.activation(logZ[:, :], sumsZ[:, :], AF.Ln)

        # log_probs = relu_logits - logZ (broadcast over V); DMA out per piece
        off = 0
        while off < n_slots_total:
            ps = min(piece_slots, n_slots_total - off)
            lp_sb = end_p.tile([128, piece_slots * V], FP32, tag="lp")
            nc.vector.tensor_tensor(
                lp_sb[:, :ps * V],
                AP(relu_ring[:, :].tensor, relu_ring[:, :].offset + off * V,
                   [list(relu_ring[:, :].ap[0]), [V, ps], [1, V]]),
                AP(logZ[:, :].tensor, logZ[:, :].offset + off,
                   [list(logZ[:, :].ap[0]), [1, ps], [0, V]]),
                ALU.subtract,
            )
            # slots are (t*NB + q); pieces are multiples of NB
            assert off % NB == 0 and (ps % NB == 0 or off + ps == n_slots_total)
            tcnt = ps // NB
            tstart = off // NB
            for q in range(NB):
                nc.sync.dma_start(
                    AP(lp_out.tensor, tstart * V + q * BC * nstep * V,
                       [[nstep * V, 128], [V, tcnt], [1, V]]),
                    AP(lp_sb[:, :].tensor, lp_sb[:, :].offset + q * V,
                       [list(lp_sb[:, :].ap[0]), [NB * V, tcnt], [1, V]]),
                )
            off += ps

        # mask[p, (t, q)] = (x[q*128+p, t+1] != IGNORE)
        mask = ring_p.tile([128, nstep * NB], FP32, tag="mask")
        nc.vector.tensor_scalar(
            mask[:, :],
            AP(xB[:, :, :].tensor, xB[:, :, :].offset + 1,
               [list(xB[:, :, :].ap[0]), [1, nstep], [T, NB]]),
            float(IGNORE), None, ALU.not_equal)

        # nll contribution per slot: (logZ - sel) * mask, then reduce over t
        nll_slot = ring_p.tile([128, nstep * NB], FP32, tag="nll_slot")
        nc.vector.tensor_tensor(
            nll_slot[:, :], logZ[:, :], sel_ring[:, :], ALU.subtract)
        nc.vector.tensor_tensor(
            nll_slot[:, :], nll_slot[:, :], mask[:, :], ALU.mult)
        nllt = end_p.tile([128, NB], FP32, tag="nllt")
        nc.vector.tensor_reduce(
            out=nllt[:, :],
            in_=AP(nll_slot[:, :].tensor, nll_slot[:, :].offset,
                   [list(nll_slot[:, :].ap[0]), [1, NB], [NB, nstep]]),
            axis=mybir.AxisListType.X,
            op=ALU.add,
        )
        nc.sync.dma_start(nll_out[:, :], nllt[:, :])

    nc.compile()
    return nc


_PROGRAM = None


def _get_program():
    global _PROGRAM
    if _PROGRAM is None:
        _PROGRAM = build_program(NCORES)
    return _PROGRAM


def host_prep(x, emb_table, w_ih, w_hh, b_ih, b_fc, b_hh, w_fc):
    """Fold weights and build the per-core input maps."""
    x = np.asarray(x).astype(np.int32)
    emb_table = np.asarray(emb_table, dtype=np.float32)
    W2 = emb_table @ np.asarray(w_ih, np.float32).T \
        + np.asarray(b_ih, np.float32) + np.asarray(b_hh, np.float32)
    whhT = np.ascontiguousarray(np.asarray(w_hh, np.float32).T)
    wfcT = np.ascontiguousarray(np.asarray(w_fc, np.float32).T)
    bfc = np.asarray(b_fc, np.float32)

    in_maps = []
    for c in range(NCORES):
        xs = x[c * B:(c + 1) * B]  # [256, 128]
        xf = xs.reshape(-1)
        idx16 = np.zeros((16, NTOK // 16), np.int16)
        ii = np.arange(NTOK)
        idx16[ii % 16, ii // 16] = xf.astype(np.int16)
        idx16 = np.tile(idx16, (8, 1))
        in_maps.append({
            "xidx16": idx16,
            "xflat32": np.ascontiguousarray(xf.astype(np.float32)),
            "xT": np.ascontiguousarray(xs.T.astype(np.float32)),
            "whhT": whhT,
            "W2": np.ascontiguousarray(W2, dtype=np.float32),
            "wfcT": wfcT,
            "bfc": bfc,
            "etab": emb_table,
        })
    return x, in_maps


def assemble(x, results):
    """Gather per-core outputs into the full-shape reference outputs."""
    lp = np.concatenate([r["lp_out"] for r in results], axis=0)
    emb = np.concatenate(
        [r["emb_out"].reshape(B, T, E) for r in results], axis=0)
    nll = np.stack([r["nll_out"] for r in results])  # [NC, 128, NB]
    # nll[c, p, q] is the masked NLL sum of batch element c*B + q*128 + p
    nll_sum = nll.transpose(0, 2, 1).reshape(B_FULL)
    lengths = (x != IGNORE).sum(axis=1).astype(np.float32)
    sample_loss = nll_sum / lengths
    mean_loss = np.float32(sample_loss.mean())
    return lp, emb, sample_loss, mean_loss


def kernel(x, emb_table, w_ih, w_hh, b_ih, b_fc, b_hh, w_fc):
    from concourse.bass_utils import run_bass_kernel_spmd

    x, in_maps = host_prep(x, emb_table, w_ih, w_hh, b_ih, b_fc, b_hh, w_fc)
    nc = _get_program()
    res = run_bass_kernel_spmd(nc, in_maps, core_ids=list(range(NCORES)))
    return assemble(x, res.results)


# revision 16
# speedup vs baseline: 1.6074x; 1.6074x over previous
"""Trainium2 Bass kernel for the AutoregressiveLSTM problem.

Data-parallel over 8 NeuronCores: batch 2048 -> 256 per core.

Per-core layout ("feature-major"): the LSTM state h and cell c live as
hT [H, B] packed into SBUF tiles [128, 2*128] (H-chunk-major), so the
recurrent matmul gatesT[4H, B] = w_hh @ h needs no per-step transposes
and the FC layer can use hT chunks directly as the stationary operand.

The input-side gate contribution xg[t] = W2[x[:, t]] (W2 = emb_table @
w_ih.T + b_ih + b_hh, folded on host) is accumulated into the same PSUM
banks via one-hot matmuls with K=36.

exp/ln (log-softmax) are deferred to a single end phase because Sigmoid/
Tanh/Relu share one ACT table set while Exp/Ln live in another (~2.7us
per table switch if interleaved).

The emb output is a pure gather -> indirect DMA, no compute engines.
"""

import math
import os
from contextlib import ExitStack

import numpy as np

import concourse.bass as bass
import concourse.tile as tile
from concourse import bacc, mybir
from concourse.bass import AP

FP32 = mybir.dt.float32
FP16 = mybir.dt.float16
I32 = mybir.dt.int32
AF = mybir.ActivationFunctionType
ALU = mybir.AluOpType

# Problem constants (hardcoded per contract)
B_FULL, T, V, E, H = 2048, 128, 36, 128, 256
NCORES = 8
B = B_FULL // NCORES          # 256 per core
NB = 2                        # batch chains per core
BC = B // NB                  # 128 per chain
G4 = 4 * H                    # 1024
NSTEP = T - 1                 # 127 LSTM steps
CH = 7                        # steps per chunk (14 FC slots of 36 = 504 <= 512 psum bank)
NTOK = B * T                  # 32768 tokens per core (for emb gather)
IGNORE = 35

# gatesT PSUM col-block m covers gate rows R[m] (PyTorch gate order i,f,g,o
# in the 4H dim). Block packing: bank0 = [i0 i1 f0 f1], bank1 = [o0 o1 g0 g1]
# so sigmoid covers cols 0:768 and tanh covers cols 768:1024.
BLOCK_ROWS = [
    (0, 128), (128, 256),          # i
    (256, 384), (384, 512),        # f
    (768, 896), (896, 1024),       # o
    (512, 640), (640, 768),        # g
]

EMB_GCHUNK = 1024              # tokens per dma_gather call (desc-ring limit)
N_EMB_G = NTOK // EMB_GCHUNK   # 32
EMB_ROWS = EMB_GCHUNK // 128   # 8 gathered rows per out free slot


def _bc(ap: AP, dims) -> AP:
    """Build a broadcast/strided view of an AP: dims = [(step, count), ...]
    appended after the partition dim. step refers to the flat free offset of
    the underlying tile."""
    part = ap.ap[0]
    return AP(ap.tensor, ap.offset, [list(part)] + [[s, c] for s, c in dims])


def build_program(num_devices: int = NCORES, nstep: int = NSTEP):
    nc = bacc.Bacc(
        "TRN2",
        target_bir_lowering=False,
        debug=False,
        enable_asserts=False,
        num_devices=num_devices,
    )

    # ---- DRAM tensors ----
    xidx16 = nc.dram_tensor("xidx16", [128, NTOK // 16], mybir.dt.int16,
                            kind="ExternalInput").ap()
    xflat32 = nc.dram_tensor("xflat32", [NTOK], FP32, kind="ExternalInput").ap()
    xT = nc.dram_tensor("xT", [T, B], FP32, kind="ExternalInput").ap()
    whhT_d = nc.dram_tensor("whhT", [H, G4], FP16, kind="ExternalInput").ap()
    W2_d = nc.dram_tensor("W2", [V, G4], FP16, kind="ExternalInput").ap()
    wfcT_d = nc.dram_tensor("wfcT", [H, V], FP16, kind="ExternalInput").ap()
    bfc_d = nc.dram_tensor("bfc", [V], FP32, kind="ExternalInput").ap()
    etab_d = nc.dram_tensor("etab", [V, E], FP32, kind="ExternalInput").ap()

    emb_out = nc.dram_tensor("emb_out", [NTOK, E], FP32, kind="ExternalOutput").ap()
    lp_out = nc.dram_tensor("lp_out", [B, nstep, V], FP32, kind="ExternalOutput").ap()
    nll_out = nc.dram_tensor("nll_out", [128, NB], FP32, kind="ExternalOutput").ap()

    with tile.TileContext(nc) as tc, ExitStack() as ctx:
        const_p = ctx.enter_context(tc.tile_pool(name="const", bufs=1))
        ring_p = ctx.enter_context(tc.tile_pool(name="ring", bufs=1))
        state_p = ctx.enter_context(tc.tile_pool(name="state", bufs=2))
        work_p = ctx.enter_context(tc.tile_pool(name="work", bufs=2))
        oh_p = ctx.enter_context(tc.tile_pool(name="oh", bufs=2))
        emb_p = ctx.enter_context(tc.tile_pool(name="emb", bufs=2))
        end_p = ctx.enter_context(tc.tile_pool(name="end", bufs=2))
        gates_pp = ctx.enter_context(tc.tile_pool(name="gates", bufs=1, space="PSUM"))
        fc_pp = ctx.enter_context(tc.tile_pool(name="fc", bufs=2, space="PSUM"))

        # ---- static loads ----
        whh_sb = []
        for j in range(2):
            wt = const_p.tile([128, G4], FP16, tag=f"whh{j}")
            nc.sync.dma_start(wt[:, :], whhT_d[j * 128:(j + 1) * 128, :])
            whh_sb.append(wt)
        W2_sb = const_p.tile([V, G4], FP16, tag="w2")
        nc.sync.dma_start(W2_sb[:, :], W2_d[:, :])
        wfc_sb = []
        for j in range(2):
            wt = const_p.tile([128, V], FP16, tag=f"wfc{j}")
            nc.sync.dma_start(wt[:, :], wfcT_d[j * 128:(j + 1) * 128, :])
            wfc_sb.append(wt)
        etab_sb = const_p.tile([V, E], FP32, tag="etab")
        nc.sync.dma_start(etab_sb[:, :], etab_d[:, :])

        bfc_tile = const_p.tile([128, 2 * CH * V], FP32, tag="bfc")
        nc.sync.dma_start(
            bfc_tile[:, :],
            AP(bfc_d.tensor, 0, [[0, 128], [0, 2 * CH], [1, V]]),
        )

        iota36 = const_p.tile([V, 1], FP32, tag="iota36")
        nc.gpsimd.iota(iota36[:, :], pattern=[[0, 1]], base=0,
                       channel_multiplier=1,
                       allow_small_or_imprecise_dtypes=True)
        iota_row = const_p.tile([128, V], FP32, tag="iotar")
        nc.gpsimd.iota(iota_row[:, :], pattern=[[1, V]], base=0,
                       channel_multiplier=0,
                       allow_small_or_imprecise_dtypes=True)

        # x in batch-chain layout: xB[p, q, t] = x[q*128+p, t]
        xB = const_p.tile([128, NB, T], FP32, tag="xB")
        nc.sync.dma_start(
            xB[:, :, :],
            AP(xflat32.tensor, 0, [[T, 128], [128 * T, NB], [1, T]]),
        )

        # rings kept for the whole scan
        relu_ring = ring_p.tile([128, nstep * NB * V], FP32, tag="relu_ring")
        sel_ring = ring_p.tile([128, nstep * NB], FP32, tag="sel_ring")

        # ---- emb output via dma_gather (independent of the scan) ----
        idx16_sb = const_p.tile([128, NTOK // 16], mybir.dt.int16, tag="idx16")
        nc.sync.dma_start(idx16_sb[:, :], xidx16[:, :])
        npc = EMB_GCHUNK // 16     # idx columns per gather chunk
        for gi in range(N_EMB_G):
            gbuf = emb_p.tile([128, EMB_ROWS, E], FP32, tag="gbuf")
            nc.gpsimd.dma_gather(
                out_ap=gbuf[:, :, :],
                in_ap=etab_d[:, :],
                idxs_ap=idx16_sb[:, gi * npc:(gi + 1) * npc],
                num_idxs=EMB_GCHUNK,
                num_idxs_reg=EMB_GCHUNK,
                elem_size=E,
            )
            nc.sync.dma_start(
                AP(emb_out.tensor, gi * EMB_GCHUNK * E,
                   [[E, 128], [128 * E, EMB_ROWS], [1, E]]),
                gbuf[:, :, :],
            )

        # ---- the scan ----
        h_prev = [None, None]   # per chain: SBUF tile [128, 256] (hT chunks)
        c_prev = [None, None]   # per chain: SBUF tile [128, 256]

        n_chunks = math.ceil(nstep / CH)
        for ci in range(n_chunks):
            t0 = ci * CH
            t1 = min(t0 + CH, nstep)
            nch = t1 - t0
            nslot = nch * NB

            # one-hot (vocab-major) for xg: ohT[v, (t_rel, b)] = (x[b, t]==v)
            xbc = oh_p.tile([V, nch * B], FP32, tag="xbc")
            nc.sync.dma_start(
                xbc[:, :],
                AP(xT.tensor, t0 * B, [[0, V], [B, nch], [1, B]]),
            )
            ohT = oh_p.tile([V, nch * B], FP16, tag="ohT")
            nc.vector.tensor_scalar(
                ohT[:, :], xbc[:, :], iota36[:, :1], None, ALU.is_equal)

            # one-hot (token-major) of targets for the NLL selection:
            # ohN[p, (t_rel, q, v)] = (x[q*128+p, t+1] == v)
            ohN = oh_p.tile([128, nslot * V], FP32, tag="ohN")
            xb_ap = xB[:, :, :]
            nc.vector.tensor_tensor(
                ohN[:, :],
                AP(xb_ap.tensor, xb_ap.offset + t0 + 1,
                   [list(xb_ap.ap[0]), [1, nch], [T, NB], [0, V]]),
                _bc(iota_row[:, :], [(0, nch), (0, NB), (1, V)]),
                ALU.is_equal,
            )

            fc_ps = fc_pp.tile([128, nslot * V], FP32, tag="fcps")

            for tr in range(nch):
                t = t0 + tr
                for q in range(NB):
                    gates = gates_pp.tile([128, G4], FP32, tag=f"gates{q}")
                    # input-side one-hot matmuls (K=36), first MM per bank
                    # clears it (start=True)
                    for m in range(8):
                        r0, r1 = BLOCK_ROWS[m]
                        nc.tensor.matmul(
                            gates[:, m * 128:(m + 1) * 128],
                            W2_sb[:, r0:r1],
                            ohT[:, tr * B + q * BC: tr * B + q * BC + BC],
                            start=(m % 4 == 0),
                            stop=(t == 0 and m % 4 == 3),
                        )
                    # recurrent matmuls
                    if t > 0:
                        for m in range(8):
                            r0, r1 = BLOCK_ROWS[m]
                            for j in range(2):
                                nc.tensor.matmul(
                                    gates[:, m * 128:(m + 1) * 128],
                                    whh_sb[j][:, r0:r1],
                                    h_prev[q][:, j * 128:(j + 1) * 128],
                                    start=False,
                                    stop=(m % 4 == 3 and j == 1),
                                )

                    ifo = work_p.tile([128, 768], FP32, tag=f"ifo{q}")
                    nc.scalar.activation(ifo[:, :], gates[:, 0:768], AF.Sigmoid)
                    gg = work_p.tile([128, 256], FP32, tag=f"g{q}")
                    nc.scalar.activation(gg[:, :], gates[:, 768:1024], AF.Tanh)

                    ig = work_p.tile([128, 256], FP32, tag=f"ig{q}")
                    nc.vector.tensor_tensor(
                        ig[:, :], ifo[:, 0:256], gg[:, :], ALU.mult)

                    if t == 0:
                        c_new = ig
                    else:
                        fcs = work_p.tile([128, 256], FP32, tag=f"fc{q}")
                        nc.gpsimd.tensor_tensor(
                            fcs[:, :], ifo[:, 256:512], c_prev[q][:, :], ALU.mult)
                        c_new = state_p.tile([128, 256], FP32, tag=f"c{q}")
                        nc.vector.tensor_tensor(
                            c_new[:, :], ig[:, :], fcs[:, :], ALU.add)
                    c_prev[q] = c_new

                    th = work_p.tile([128, 256], FP32, tag=f"th{q}")
                    nc.scalar.activation(th[:, :], c_new[:, :], AF.Tanh)
                    h_new = state_p.tile([128, 256], FP16, tag=f"h{q}")
                    nc.vector.tensor_tensor(
                        h_new[:, :], ifo[:, 512:768], th[:, :], ALU.mult)
                    h_prev[q] = h_new

                    # FC: logits[tokens, V] for this (t, q) into its fc slot
                    slot = tr * NB + q
                    for j in range(2):
                        nc.tensor.matmul(
                            fc_ps[:, slot * V:(slot + 1) * V],
                            h_new[:, j * 128:(j + 1) * 128],
                            wfc_sb[j][:, :],
                            start=(slot == 0 and j == 0),
                            stop=(slot == nslot - 1 and j == 1),
                        )

            # chunk epilogue: bias, relu into the ring, NLL selection
            biased = work_p.tile([128, nslot * V], FP32, tag="biased")
            nc.vector.tensor_tensor(
                biased[:, :], fc_ps[:, :], bfc_tile[:, :nslot * V], ALU.add)
            nc.scalar.activation(
                relu_ring[:, t0 * NB * V:(t0 * NB + nslot) * V],
                biased[:, :], AF.Relu)
            selp = work_p.tile([128, nslot * V], FP32, tag="selp")
            rr_ap = relu_ring[:, :]
            nc.gpsimd.tensor_tensor(
                selp[:, :],
                AP(rr_ap.tensor, rr_ap.offset + t0 * NB * V,
                   [list(rr_ap.ap[0]), [1, nslot * V]]),
                ohN[:, :], ALU.mult)
            nc.vector.tensor_reduce(
                out=sel_ring[:, t0 * NB:t0 * NB + nslot],
                in_=AP(selp[:, :].tensor, selp[:, :].offset,
                       [list(selp[:, :].ap[0]), [V, nslot], [1, V]]),
                axis=mybir.AxisListType.X,
                op=ALU.add,
            )

        # ---- end phase: exp/ln (one ACT table switch), log-probs, NLL ----
        logZ = ring_p.tile([128, nstep * NB], FP32, tag="logZ")
        n_slots_total = nstep * NB
        piece_slots = 64
        off = 0
        while off < n_slots_total:
            ps = min(piece_slots, n_slots_total - off)
            expx = end_p.tile([128, piece_slots * V], FP32, tag="expx")
            nc.scalar.activation(
                expx[:, :ps * V],
                relu_ring[:, off * V:(off + ps) * V], AF.Exp)
            nc.vector.tensor_reduce(
                out=logZ[:, off:off + ps],
                in_=AP(expx[:, :].tensor, expx[:, :].offset,
                       [list(expx[:, :].ap[0]), [V, ps], [1, V]]),
                axis=mybir.AxisListType.X,
                op=ALU.add,
            )
            off += ps
        sumsZ = logZ
        logZ = ring_p.tile([128, nstep * NB], FP32, tag="logZ2")
        nc.scalar.activation(logZ[:, :], sumsZ[:, :], AF.Ln)

        # log_probs = relu_logits - logZ (broadcast over V); DMA out per piece
        off = 0
        while off < n_slots_total:
            ps = min(piece_slots, n_slots_total - off)
            lp_sb = end_p.tile([128, piece_slots * V], FP32, tag="lp")
            nc.vector.tensor_tensor(
                lp_sb[:, :ps * V],
                AP(relu_ring[:, :].tensor, relu_ring[:, :].offset + off * V,
                   [list(relu_ring[:, :].ap[0]), [V, ps], [1, V]]),
                AP(logZ[:, :].tensor, logZ[:, :].offset + off,
                   [list(logZ[:, :].ap[0]), [1, ps], [0, V]]),
                ALU.subtract,
            )
            # slots are (t*NB + q); pieces are multiples of NB
            assert off % NB == 0 and (ps % NB == 0 or off + ps == n_slots_total)
            tcnt = ps // NB
            tstart = off // NB
            for q in range(NB):
                nc.sync.dma_start(
                    AP(lp_out.tensor, tstart * V + q * BC * nstep * V,
                       [[nstep * V, 128], [V, tcnt], [1, V]]),
                    AP(lp_sb[:, :].tensor, lp_sb[:, :].offset + q * V,
                       [list(lp_sb[:, :].ap[0]), [NB * V, tcnt], [1, V]]),
                )
            off += ps

        # mask[p, (t, q)] = (x[q*128+p, t+1] != IGNORE)
        mask = ring_p.tile([128, nstep * NB], FP32, tag="mask")
        nc.vector.tensor_scalar(
            mask[:, :],
            AP(xB[:, :, :].tensor, xB[:, :, :].offset + 1,
               [list(xB[:, :, :].ap[0]), [1, nstep], [T, NB]]),
            float(IGNORE), None, ALU.not_equal)

        # nll contribution per slot: (logZ - sel) * mask, then reduce over t
        nll_slot = ring_p.tile([128, nstep * NB], FP32, tag="nll_slot")
        nc.vector.tensor_tensor(
            nll_slot[:, :], logZ[:, :], sel_ring[:, :], ALU.subtract)
        nc.vector.tensor_tensor(
            nll_slot[:, :], nll_slot[:, :], mask[:, :], ALU.mult)
        nllt = end_p.tile([128, NB], FP32, tag="nllt")
        nc.vector.tensor_reduce(
            out=nllt[:, :],
            in_=AP(nll_slot[:, :].tensor, nll_slot[:, :].offset,
                   [list(nll_slot[:, :].ap[0]), [1, NB], [NB, nstep]]),
            axis=mybir.AxisListType.X,
            op=ALU.add,
        )
        nc.sync.dma_start(nll_out[:, :], nllt[:, :])

    nc.compile()
    return nc


_PROGRAM = None


def _get_program():
    global _PROGRAM
    if _PROGRAM is None:
        _PROGRAM = build_program(NCORES)
    return _PROGRAM


def host_prep(x, emb_table, w_ih, w_hh, b_ih, b_fc, b_hh, w_fc):
    """Fold weights and build the per-core input maps."""
    x = np.asarray(x).astype(np.int32)
    emb_table = np.asarray(emb_table, dtype=np.float32)
    W2 = emb_table @ np.asarray(w_ih, np.float32).T \
        + np.asarray(b_ih, np.float32) + np.asarray(b_hh, np.float32)
    W2 = W2.astype(np.float16)
    whhT = np.ascontiguousarray(np.asarray(w_hh, np.float32).T.astype(np.float16))
    wfcT = np.ascontiguousarray(np.asarray(w_fc, np.float32).T.astype(np.float16))
    bfc = np.asarray(b_fc, np.float32)

    in_maps = []
    for c in range(NCORES):
        xs = x[c * B:(c + 1) * B]  # [256, 128]
        xf = xs.reshape(-1)
        idx16 = np.zeros((16, NTOK // 16), np.int16)
        ii = np.arange(NTOK)
        idx16[ii % 16, ii // 16] = xf.astype(np.int16)
        idx16 = np.tile(idx16, (8, 1))
        in_maps.append({
            "xidx16": idx16,
            "xflat32": np.ascontiguousarray(xf.astype(np.float32)),
            "xT": np.ascontiguousarray(xs.T.astype(np.float32)),
            "whhT": whhT,
            "W2": np.ascontiguousarray(W2),
            "wfcT": wfcT,
            "bfc": bfc,
            "etab": emb_table,
        })
    return x, in_maps


def assemble(x, results):
    """Gather per-core outputs into the full-shape reference outputs."""
    lp = np.concatenate([r["lp_out"] for r in results], axis=0)
    emb = np.concatenate(
        [r["emb_out"].reshape(B, T, E) for r in results], axis=0)
    nll = np.stack([r["nll_out"] for r in results])  # [NC, 128, NB]
    # nll[c, p, q] is the masked NLL sum of batch element c*B + q*128 + p
    nll_sum = nll.transpose(0, 2, 1).reshape(B_FULL)
    lengths = (x != IGNORE).sum(axis=1).astype(np.float32)
    sample_loss = nll_sum / lengths
    mean_loss = np.float32(sample_loss.mean())
    return lp, emb, sample_loss, mean_loss


def kernel(x, emb_table, w_ih, w_hh, b_ih, b_fc, b_hh, w_fc):
    from concourse.bass_utils import run_bass_kernel_spmd

    x, in_maps = host_prep(x, emb_table, w_ih, w_hh, b_ih, b_fc, b_hh, w_fc)
    nc = _get_program()
    res = run_bass_kernel_spmd(nc, in_maps, core_ids=list(range(NCORES)))
    return assemble(x, res.results)


# revision 17
# speedup vs baseline: 2.4378x; 1.5166x over previous
"""Trainium2 Bass kernel for the AutoregressiveLSTM problem.

Data-parallel over 8 NeuronCores: batch 2048 -> 256 per core.

Per-core layout ("feature-major"): the LSTM state h and cell c live as
hT [H, B] packed into SBUF tiles [128, 2*128] (H-chunk-major), so the
recurrent matmul gatesT[4H, B] = w_hh @ h needs no per-step transposes
and the FC layer can use hT chunks directly as the stationary operand.

The input-side gate contribution xg[t] = W2[x[:, t]] (W2 = emb_table @
w_ih.T + b_ih + b_hh, folded on host) is accumulated into the same PSUM
banks via one-hot matmuls with K=36.

exp/ln (log-softmax) are deferred to a single end phase because Sigmoid/
Tanh/Relu share one ACT table set while Exp/Ln live in another (~2.7us
per table switch if interleaved).

The emb output is a pure gather -> indirect DMA, no compute engines.
"""

import math
import os
from contextlib import ExitStack

import numpy as np

import concourse.bass as bass
import concourse.tile as tile
from concourse import bacc, mybir
from concourse.bass import AP

FP32 = mybir.dt.float32
FP16 = mybir.dt.float16
I32 = mybir.dt.int32
AF = mybir.ActivationFunctionType
ALU = mybir.AluOpType

# Problem constants (hardcoded per contract)
B_FULL, T, V, E, H = 2048, 128, 36, 128, 256
NCORES = 8
B = B_FULL // NCORES          # 256 per core
NB = 2                        # batch chains per core
BC = B // NB                  # 128 per chain
G4 = 4 * H                    # 1024
NSTEP = T - 1                 # 127 LSTM steps
CH = 7                        # steps per chunk (14 FC slots of 36 = 504 <= 512 psum bank)
NTOK = B * T                  # 32768 tokens per core (for emb gather)
IGNORE = 35

# gatesT PSUM col-block m covers gate rows R[m] (PyTorch gate order i,f,g,o
# in the 4H dim). Block packing: bank0 = [i0 i1 f0 f1], bank1 = [o0 o1 g0 g1]
# so sigmoid covers cols 0:768 and tanh covers cols 768:1024.
BLOCK_ROWS = [
    (0, 128), (128, 256),          # i
    (256, 384), (384, 512),        # f
    (768, 896), (896, 1024),       # o
    (512, 640), (640, 768),        # g
]

EMB_GCHUNK = 1024              # tokens per dma_gather call (desc-ring limit)
N_EMB_G = NTOK // EMB_GCHUNK   # 32
EMB_ROWS = EMB_GCHUNK // 128   # 8 gathered rows per out free slot


def _bc(ap: AP, dims) -> AP:
    """Build a broadcast/strided view of an AP: dims = [(step, count), ...]
    appended after the partition dim. step refers to the flat free offset of
    the underlying tile."""
    part = ap.ap[0]
    return AP(ap.tensor, ap.offset, [list(part)] + [[s, c] for s, c in dims])


def build_program(num_devices: int = NCORES, nstep: int = NSTEP):
    nc = bacc.Bacc(
        "TRN2",
        target_bir_lowering=False,
        debug=False,
        enable_asserts=False,
        num_devices=num_devices,
    )

    # ---- DRAM tensors ----
    xidx16 = nc.dram_tensor("xidx16", [128, NTOK // 16], mybir.dt.int16,
                            kind="ExternalInput").ap()
    xflat32 = nc.dram_tensor("xflat32", [NTOK], FP32, kind="ExternalInput").ap()
    xT = nc.dram_tensor("xT", [T, B], FP32, kind="ExternalInput").ap()
    whhT_d = nc.dram_tensor("whhT", [H, G4], FP16, kind="ExternalInput").ap()
    W2_d = nc.dram_tensor("W2", [V, G4], FP16, kind="ExternalInput").ap()
    wfcT_d = nc.dram_tensor("wfcT", [H, V], FP16, kind="ExternalInput").ap()
    bfc_d = nc.dram_tensor("bfc", [V], FP32, kind="ExternalInput").ap()
    etab_d = nc.dram_tensor("etab", [V, E], FP32, kind="ExternalInput").ap()

    emb_out = nc.dram_tensor("emb_out", [NTOK, E], FP32, kind="ExternalOutput").ap()
    lp_out = nc.dram_tensor("lp_out", [B, nstep, V], FP32, kind="ExternalOutput").ap()
    nll_out = nc.dram_tensor("nll_out", [128, NB], FP32, kind="ExternalOutput").ap()

    with tile.TileContext(nc) as tc, ExitStack() as ctx:
        const_p = ctx.enter_context(tc.tile_pool(name="const", bufs=1))
        ring_p = ctx.enter_context(tc.tile_pool(name="ring", bufs=1))
        state_p = ctx.enter_context(tc.tile_pool(name="state", bufs=2))
        work_p = ctx.enter_context(tc.tile_pool(name="work", bufs=2))
        oh_p = ctx.enter_context(tc.tile_pool(name="oh", bufs=2))
        emb_p = ctx.enter_context(tc.tile_pool(name="emb", bufs=2))
        end_p = ctx.enter_context(tc.tile_pool(name="end", bufs=2))
        gates_pp = ctx.enter_context(tc.tile_pool(name="gates", bufs=1, space="PSUM"))
        fc_pp = ctx.enter_context(tc.tile_pool(name="fc", bufs=2, space="PSUM"))

        # ---- static loads ----
        whh_sb = []
        for j in range(2):
            wt = const_p.tile([128, G4], FP16, tag=f"whh{j}")
            nc.sync.dma_start(wt[:, :], whhT_d[j * 128:(j + 1) * 128, :])
            whh_sb.append(wt)
        W2_sb = const_p.tile([V, G4], FP16, tag="w2")
        nc.sync.dma_start(W2_sb[:, :], W2_d[:, :])
        wfc_sb = []
        for j in range(2):
            wt = const_p.tile([128, V], FP16, tag=f"wfc{j}")
            nc.sync.dma_start(wt[:, :], wfcT_d[j * 128:(j + 1) * 128, :])
            wfc_sb.append(wt)
        etab_sb = const_p.tile([V, E], FP32, tag="etab")
        nc.sync.dma_start(etab_sb[:, :], etab_d[:, :])

        bfc_tile = const_p.tile([128, 2 * CH * V], FP32, tag="bfc")
        nc.sync.dma_start(
            bfc_tile[:, :],
            AP(bfc_d.tensor, 0, [[0, 128], [0, 2 * CH], [1, V]]),
        )

        iota36 = const_p.tile([V, 1], FP32, tag="iota36")
        nc.gpsimd.iota(iota36[:, :], pattern=[[0, 1]], base=0,
                       channel_multiplier=1,
                       allow_small_or_imprecise_dtypes=True)
        iota_row = const_p.tile([128, V], FP32, tag="iotar")
        nc.gpsimd.iota(iota_row[:, :], pattern=[[1, V]], base=0,
                       channel_multiplier=0,
                       allow_small_or_imprecise_dtypes=True)

        # x in batch-chain layout: xB[p, q, t] = x[q*128+p, t]
        xB = const_p.tile([128, NB, T], FP32, tag="xB")
        nc.sync.dma_start(
            xB[:, :, :],
            AP(xflat32.tensor, 0, [[T, 128], [128 * T, NB], [1, T]]),
        )

        # rings kept for the whole scan
        relu_ring = ring_p.tile([128, nstep * NB * V], FP32, tag="relu_ring")
        sel_ring = ring_p.tile([128, nstep * NB], FP32, tag="sel_ring")

        # ---- emb output via dma_gather (independent of the scan) ----
        idx16_sb = const_p.tile([128, NTOK // 16], mybir.dt.int16, tag="idx16")
        nc.sync.dma_start(idx16_sb[:, :], xidx16[:, :])
        npc = EMB_GCHUNK // 16     # idx columns per gather chunk
        for gi in range(N_EMB_G):
            gbuf = emb_p.tile([128, EMB_ROWS, E], FP32, tag="gbuf")
            nc.gpsimd.dma_gather(
                out_ap=gbuf[:, :, :],
                in_ap=etab_d[:, :],
                idxs_ap=idx16_sb[:, gi * npc:(gi + 1) * npc],
                num_idxs=EMB_GCHUNK,
                num_idxs_reg=EMB_GCHUNK,
                elem_size=E,
            )
            nc.sync.dma_start(
                AP(emb_out.tensor, gi * EMB_GCHUNK * E,
                   [[E, 128], [128 * E, EMB_ROWS], [1, E]]),
                gbuf[:, :, :],
            )

        # ---- the scan ----
        h_prev = [None, None]   # per chain: SBUF tile [128, 256] (hT chunks)
        c_prev = [None, None]   # per chain: SBUF tile [128, 256]

        n_chunks = math.ceil(nstep / CH)
        for ci in range(n_chunks):
            t0 = ci * CH
            t1 = min(t0 + CH, nstep)
            nch = t1 - t0
            nslot = nch * NB

            # one-hot (vocab-major) for xg: ohT[v, (t_rel, b)] = (x[b, t]==v)
            xbc = oh_p.tile([V, nch * B], FP32, tag="xbc")
            nc.sync.dma_start(
                xbc[:, :],
                AP(xT.tensor, t0 * B, [[0, V], [B, nch], [1, B]]),
            )
            ohT = oh_p.tile([V, nch * B], FP16, tag="ohT")
            nc.vector.tensor_scalar(
                ohT[:, :], xbc[:, :], iota36[:, :1], None, ALU.is_equal)

            # one-hot (token-major) of targets for the NLL selection:
            # ohN[p, (t_rel, q, v)] = (x[q*128+p, t+1] == v)
            ohN = oh_p.tile([128, nslot * V], FP32, tag="ohN")
            xb_ap = xB[:, :, :]
            nc.vector.tensor_tensor(
                ohN[:, :],
                AP(xb_ap.tensor, xb_ap.offset + t0 + 1,
                   [list(xb_ap.ap[0]), [1, nch], [T, NB], [0, V]]),
                _bc(iota_row[:, :], [(0, nch), (0, NB), (1, V)]),
                ALU.is_equal,
            )

            fc_ps = fc_pp.tile([128, nslot * V], FP32, tag="fcps")

            for tr in range(nch):
                t = t0 + tr
                for q in range(NB):
                    gates = gates_pp.tile([128, G4], FP32, tag=f"gates{q}")
                    # input-side one-hot matmuls (K=36), first MM per bank
                    # clears it (start=True)
                    for m in range(8):
                        r0, r1 = BLOCK_ROWS[m]
                        nc.tensor.matmul(
                            gates[:, m * 128:(m + 1) * 128],
                            W2_sb[:, r0:r1],
                            ohT[:, tr * B + q * BC: tr * B + q * BC + BC],
                            start=(m % 4 == 0),
                            stop=(t == 0 and m % 4 == 3),
                        )
                    # recurrent matmuls
                    if t > 0:
                        for m in range(8):
                            r0, r1 = BLOCK_ROWS[m]
                            for j in range(2):
                                nc.tensor.matmul(
                                    gates[:, m * 128:(m + 1) * 128],
                                    whh_sb[j][:, r0:r1],
                                    h_prev[q][:, j * 128:(j + 1) * 128],
                                    start=False,
                                    stop=(m % 4 == 3 and j == 1),
                                )

                    ifo = work_p.tile([128, 768], FP32, tag=f"ifo{q}")
                    nc.scalar.activation(ifo[:, :], gates[:, 0:768], AF.Sigmoid)
                    gg = work_p.tile([128, 256], FP32, tag=f"g{q}")
                    nc.scalar.activation(gg[:, :], gates[:, 768:1024], AF.Tanh)

                    ig = work_p.tile([128, 256], FP32, tag=f"ig{q}")
                    nc.vector.tensor_tensor(
                        ig[:, :], ifo[:, 0:256], gg[:, :], ALU.mult)

                    if t == 0:
                        c_new = ig
                    else:
                        fcs = work_p.tile([128, 256], FP32, tag=f"fc{q}")
                        nc.vector.tensor_tensor(
                            fcs[:, :], ifo[:, 256:512], c_prev[q][:, :], ALU.mult)
                        c_new = state_p.tile([128, 256], FP32, tag=f"c{q}")
                        nc.vector.tensor_tensor(
                            c_new[:, :], ig[:, :], fcs[:, :], ALU.add)
                    c_prev[q] = c_new

                    th = work_p.tile([128, 256], FP32, tag=f"th{q}")
                    nc.scalar.activation(th[:, :], c_new[:, :], AF.Tanh)
                    h_new = state_p.tile([128, 256], FP16, tag=f"h{q}")
                    nc.vector.tensor_tensor(
                        h_new[:, :], ifo[:, 512:768], th[:, :], ALU.mult)
                    h_prev[q] = h_new

                    # FC: logits[tokens, V] for this (t, q) into its fc slot
                    slot = tr * NB + q
                    for j in range(2):
                        nc.tensor.matmul(
                            fc_ps[:, slot * V:(slot + 1) * V],
                            h_new[:, j * 128:(j + 1) * 128],
                            wfc_sb[j][:, :],
                            start=(slot == 0 and j == 0),
                            stop=(slot == nslot - 1 and j == 1),
                        )

            # chunk epilogue: bias, relu into the ring, NLL selection
            biased = work_p.tile([128, nslot * V], FP32, tag="biased")
            nc.vector.tensor_tensor(
                biased[:, :], fc_ps[:, :], bfc_tile[:, :nslot * V], ALU.add)
            nc.scalar.activation(
                relu_ring[:, t0 * NB * V:(t0 * NB + nslot) * V],
                biased[:, :], AF.Relu)
            selp = work_p.tile([128, nslot * V], FP32, tag="selp")
            rr_ap = relu_ring[:, :]
            nc.vector.tensor_tensor(
                selp[:, :],
                AP(rr_ap.tensor, rr_ap.offset + t0 * NB * V,
                   [list(rr_ap.ap[0]), [1, nslot * V]]),
                ohN[:, :], ALU.mult)
            nc.vector.tensor_reduce(
                out=sel_ring[:, t0 * NB:t0 * NB + nslot],
                in_=AP(selp[:, :].tensor, selp[:, :].offset,
                       [list(selp[:, :].ap[0]), [V, nslot], [1, V]]),
                axis=mybir.AxisListType.X,
                op=ALU.add,
            )

        # ---- end phase: exp/ln (one ACT table switch), log-probs, NLL ----
        logZ = ring_p.tile([128, nstep * NB], FP32, tag="logZ")
        n_slots_total = nstep * NB
        piece_slots = 64
        off = 0
        while off < n_slots_total:
            ps = min(piece_slots, n_slots_total - off)
            expx = end_p.tile([128, piece_slots * V], FP32, tag="expx")
            nc.scalar.activation(
                expx[:, :ps * V],
                relu_ring[:, off * V:(off + ps) * V], AF.Exp)
            nc.vector.tensor_reduce(
                out=logZ[:, off:off + ps],
                in_=AP(expx[:, :].tensor, expx[:, :].offset,
                       [list(expx[:, :].ap[0]), [V, ps], [1, V]]),
                axis=mybir.AxisListType.X,
                op=ALU.add,
            )
            off += ps
        sumsZ = logZ
        logZ = ring_p.tile([128, nstep * NB], FP32, tag="logZ2")
        nc.scalar.activation(logZ[:, :], sumsZ[:, :], AF.Ln)

        # log_probs = relu_logits - logZ (broadcast over V); DMA out per piece
        off = 0
        while off < n_slots_total:
            ps = min(piece_slots, n_slots_total - off)
            lp_sb = end_p.tile([128, piece_slots * V], FP32, tag="lp")
            nc.vector.tensor_tensor(
                lp_sb[:, :ps * V],
                AP(relu_ring[:, :].tensor, relu_ring[:, :].offset + off * V,
                   [list(relu_ring[:, :].ap[0]), [V, ps], [1, V]]),
                AP(logZ[:, :].tensor, logZ[:, :].offset + off,
                   [list(logZ[:, :].ap[0]), [1, ps], [0, V]]),
                ALU.subtract,
            )
            # slots are (t*NB + q); pieces are multiples of NB
            assert off % NB == 0 and (ps % NB == 0 or off + ps == n_slots_total)
            tcnt = ps // NB
            tstart = off // NB
            for q in range(NB):
                nc.sync.dma_start(
                    AP(lp_out.tensor, tstart * V + q * BC * nstep * V,
                       [[nstep * V, 128], [V, tcnt], [1, V]]),
                    AP(lp_sb[:, :].tensor, lp_sb[:, :].offset + q * V,
                       [list(lp_sb[:, :].ap[0]), [NB * V, tcnt], [1, V]]),
                )
            off += ps

        # mask[p, (t, q)] = (x[q*128+p, t+1] != IGNORE)
        mask = ring_p.tile([128, nstep * NB], FP32, tag="mask")
        nc.vector.tensor_scalar(
            mask[:, :],
            AP(xB[:, :, :].tensor, xB[:, :, :].offset + 1,
               [list(xB[:, :, :].ap[0]), [1, nstep], [T, NB]]),
            float(IGNORE), None, ALU.not_equal)

        # nll contribution per slot: (logZ - sel) * mask, then reduce over t
        nll_slot = ring_p.tile([128, nstep * NB], FP32, tag="nll_slot")
        nc.vector.tensor_tensor(
            nll_slot[:, :], logZ[:, :], sel_ring[:, :], ALU.subtract)
        nc.vector.tensor_tensor(
            nll_slot[:, :], nll_slot[:, :], mask[:, :], ALU.mult)
        nllt = end_p.tile([128, NB], FP32, tag="nllt")
        nc.vector.tensor_reduce(
            out=nllt[:, :],
            in_=AP(nll_slot[:, :].tensor, nll_slot[:, :].offset,
                   [list(nll_slot[:, :].ap[0]), [1, NB], [NB, nstep]]),
            axis=mybir.AxisListType.X,
            op=ALU.add,
        )
        nc.sync.dma_start(nll_out[:, :], nllt[:, :])

    nc.compile()
    return nc


_PROGRAM = None


def _get_program():
    global _PROGRAM
    if _PROGRAM is None:
        _PROGRAM = build_program(NCORES)
    return _PROGRAM


def host_prep(x, emb_table, w_ih, w_hh, b_ih, b_fc, b_hh, w_fc):
    """Fold weights and build the per-core input maps."""
    x = np.asarray(x).astype(np.int32)
    emb_table = np.asarray(emb_table, dtype=np.float32)
    W2 = emb_table @ np.asarray(w_ih, np.float32).T \
        + np.asarray(b_ih, np.float32) + np.asarray(b_hh, np.float32)
    W2 = W2.astype(np.float16)
    whhT = np.ascontiguousarray(np.asarray(w_hh, np.float32).T.astype(np.float16))
    wfcT = np.ascontiguousarray(np.asarray(w_fc, np.float32).T.astype(np.float16))
    bfc = np.asarray(b_fc, np.float32)

    in_maps = []
    for c in range(NCORES):
        xs = x[c * B:(c + 1) * B]  # [256, 128]
        xf = xs.reshape(-1)
        idx16 = np.zeros((16, NTOK // 16), np.int16)
        ii = np.arange(NTOK)
        idx16[ii % 16, ii // 16] = xf.astype(np.int16)
        idx16 = np.tile(idx16, (8, 1))
        in_maps.append({
            "xidx16": idx16,
            "xflat32": np.ascontiguousarray(xf.astype(np.float32)),
            "xT": np.ascontiguousarray(xs.T.astype(np.float32)),
            "whhT": whhT,
            "W2": np.ascontiguousarray(W2),
            "wfcT": wfcT,
            "bfc": bfc,
            "etab": emb_table,
        })
    return x, in_maps


def assemble(x, results):
    """Gather per-core outputs into the full-shape reference outputs."""
    lp = np.concatenate([r["lp_out"] for r in results], axis=0)
    emb = np.concatenate(
        [r["emb_out"].reshape(B, T, E) for r in results], axis=0)
    nll = np.stack([r["nll_out"] for r in results])  # [NC, 128, NB]
    # nll[c, p, q] is the masked NLL sum of batch element c*B + q*128 + p
    nll_sum = nll.transpose(0, 2, 1).reshape(B_FULL)
    lengths = (x != IGNORE).sum(axis=1).astype(np.float32)
    sample_loss = nll_sum / lengths
    mean_loss = np.float32(sample_loss.mean())
    return lp, emb, sample_loss, mean_loss


def kernel(x, emb_table, w_ih, w_hh, b_ih, b_fc, b_hh, w_fc):
    from concourse.bass_utils import run_bass_kernel_spmd

    x, in_maps = host_prep(x, emb_table, w_ih, w_hh, b_ih, b_fc, b_hh, w_fc)
    nc = _get_program()
    res = run_bass_kernel_spmd(nc, in_maps, core_ids=list(range(NCORES)))
    return assemble(x, res.results)


# revision 18
# speedup vs baseline: 2.5614x; 1.0507x over previous
"""Trainium2 Bass kernel for the AutoregressiveLSTM problem.

Data-parallel over 8 NeuronCores: batch 2048 -> 256 per core.

Per-core layout ("feature-major"): the LSTM state h and cell c live as
hT [H, B] packed into SBUF tiles [128, 2*128] (H-chunk-major), so the
recurrent matmul gatesT[4H, B] = w_hh @ h needs no per-step transposes
and the FC layer can use hT chunks directly as the stationary operand.

The input-side gate contribution xg[t] = W2[x[:, t]] (W2 = emb_table @
w_ih.T + b_ih + b_hh, folded on host) is accumulated into the same PSUM
banks via one-hot matmuls with K=36.

exp/ln (log-softmax) are deferred to a single end phase because Sigmoid/
Tanh/Relu share one ACT table set while Exp/Ln live in another (~2.7us
per table switch if interleaved).

The emb output is a pure gather -> indirect DMA, no compute engines.
"""

import math
import os
from contextlib import ExitStack

import numpy as np

import concourse.bass as bass
import concourse.tile as tile
from concourse import bacc, mybir
from concourse.bass import AP

FP32 = mybir.dt.float32
FP16 = mybir.dt.float16
I32 = mybir.dt.int32
AF = mybir.ActivationFunctionType
ALU = mybir.AluOpType

# Problem constants (hardcoded per contract)
B_FULL, T, V, E, H = 2048, 128, 36, 128, 256
NCORES = 8
B = B_FULL // NCORES          # 256 per core
NB = 2                        # batch chains per core
BC = B // NB                  # 128 per chain
G4 = 4 * H                    # 1024
NSTEP = T - 1                 # 127 LSTM steps
CH = 7                        # steps per chunk (14 FC slots of 36 = 504 <= 512 psum bank)
NTOK = B * T                  # 32768 tokens per core (for emb gather)
IGNORE = 35

# gatesT PSUM col-block m covers gate rows R[m] (PyTorch gate order i,f,g,o
# in the 4H dim). Block packing: bank0 = [i0 i1 f0 f1], bank1 = [o0 o1 g0 g1]
# so sigmoid covers cols 0:768 and tanh covers cols 768:1024.
BLOCK_ROWS = [
    (0, 128), (128, 256),          # i
    (256, 384), (384, 512),        # f
    (768, 896), (896, 1024),       # o
    (512, 640), (640, 768),        # g
]

EMB_GCHUNK = 1024              # tokens per dma_gather call (desc-ring limit)
N_EMB_G = NTOK // EMB_GCHUNK   # 32
EMB_ROWS = EMB_GCHUNK // 128   # 8 gathered rows per out free slot


def _bc(ap: AP, dims) -> AP:
    """Build a broadcast/strided view of an AP: dims = [(step, count), ...]
    appended after the partition dim. step refers to the flat free offset of
    the underlying tile."""
    part = ap.ap[0]
    return AP(ap.tensor, ap.offset, [list(part)] + [[s, c] for s, c in dims])


def build_program(num_devices: int = NCORES, nstep: int = NSTEP):
    nc = bacc.Bacc(
        "TRN2",
        target_bir_lowering=False,
        debug=False,
        enable_asserts=False,
        num_devices=num_devices,
    )

    # ---- DRAM tensors ----
    xidx16 = nc.dram_tensor("xidx16", [128, NTOK // 16], mybir.dt.int16,
                            kind="ExternalInput").ap()
    xflat32 = nc.dram_tensor("xflat32", [NTOK], FP32, kind="ExternalInput").ap()
    xT = nc.dram_tensor("xT", [T, B], FP32, kind="ExternalInput").ap()
    whhT_d = nc.dram_tensor("whhT", [H, G4], FP16, kind="ExternalInput").ap()
    W2_d = nc.dram_tensor("W2", [V, G4], FP16, kind="ExternalInput").ap()
    wfcT_d = nc.dram_tensor("wfcT", [H, V], FP16, kind="ExternalInput").ap()
    bfc_d = nc.dram_tensor("bfc", [V], FP32, kind="ExternalInput").ap()
    etab_d = nc.dram_tensor("etab", [V, E], FP32, kind="ExternalInput").ap()

    emb_out = nc.dram_tensor("emb_out", [NTOK, E], FP32, kind="ExternalOutput").ap()
    lp_out = nc.dram_tensor("lp_out", [B, nstep, V], FP32, kind="ExternalOutput").ap()
    nll_out = nc.dram_tensor("nll_out", [128, NB], FP32, kind="ExternalOutput").ap()

    with tile.TileContext(nc) as tc, ExitStack() as ctx:
        const_p = ctx.enter_context(tc.tile_pool(name="const", bufs=1))
        ring_p = ctx.enter_context(tc.tile_pool(name="ring", bufs=1))
        state_p = ctx.enter_context(tc.tile_pool(name="state", bufs=2))
        work_p = ctx.enter_context(tc.tile_pool(name="work", bufs=2))
        oh_p = ctx.enter_context(tc.tile_pool(name="oh", bufs=2))
        emb_p = ctx.enter_context(tc.tile_pool(name="emb", bufs=2))
        end_p = ctx.enter_context(tc.tile_pool(name="end", bufs=2))
        gates_pp = ctx.enter_context(tc.tile_pool(name="gates", bufs=1, space="PSUM"))
        fc_pp = ctx.enter_context(tc.tile_pool(name="fc", bufs=2, space="PSUM"))

        # ---- static loads ----
        whh_sb = []
        for j in range(2):
            wt = const_p.tile([128, G4], FP16, tag=f"whh{j}")
            nc.sync.dma_start(wt[:, :], whhT_d[j * 128:(j + 1) * 128, :])
            whh_sb.append(wt)
        W2_sb = const_p.tile([V, G4], FP16, tag="w2")
        nc.sync.dma_start(W2_sb[:, :], W2_d[:, :])
        wfc_sb = []
        for j in range(2):
            wt = const_p.tile([128, V], FP16, tag=f"wfc{j}")
            nc.sync.dma_start(wt[:, :], wfcT_d[j * 128:(j + 1) * 128, :])
            wfc_sb.append(wt)
        etab_sb = const_p.tile([V, E], FP32, tag="etab")
        nc.sync.dma_start(etab_sb[:, :], etab_d[:, :])

        bfc_tile = const_p.tile([128, 2 * CH * V], FP32, tag="bfc")
        nc.sync.dma_start(
            bfc_tile[:, :],
            AP(bfc_d.tensor, 0, [[0, 128], [0, 2 * CH], [1, V]]),
        )

        iota36 = const_p.tile([V, 1], FP32, tag="iota36")
        nc.gpsimd.iota(iota36[:, :], pattern=[[0, 1]], base=0,
                       channel_multiplier=1,
                       allow_small_or_imprecise_dtypes=True)
        iota_row = const_p.tile([128, V], FP32, tag="iotar")
        nc.gpsimd.iota(iota_row[:, :], pattern=[[1, V]], base=0,
                       channel_multiplier=0,
                       allow_small_or_imprecise_dtypes=True)

        # x in batch-chain layout: xB[p, q, t] = x[q*128+p, t]
        xB = const_p.tile([128, NB, T], FP32, tag="xB")
        nc.sync.dma_start(
            xB[:, :, :],
            AP(xflat32.tensor, 0, [[T, 128], [128 * T, NB], [1, T]]),
        )

        # rings kept for the whole scan
        relu_ring = ring_p.tile([128, nstep * NB * V], FP32, tag="relu_ring")
        sel_ring = ring_p.tile([128, nstep * NB], FP32, tag="sel_ring")

        # ---- emb output via dma_gather (independent of the scan) ----
        idx16_sb = const_p.tile([128, NTOK // 16], mybir.dt.int16, tag="idx16")
        nc.sync.dma_start(idx16_sb[:, :], xidx16[:, :])
        npc = EMB_GCHUNK // 16     # idx columns per gather chunk
        for gi in range(N_EMB_G):
            gbuf = emb_p.tile([128, EMB_ROWS, E], FP32, tag="gbuf")
            nc.gpsimd.dma_gather(
                out_ap=gbuf[:, :, :],
                in_ap=etab_d[:, :],
                idxs_ap=idx16_sb[:, gi * npc:(gi + 1) * npc],
                num_idxs=EMB_GCHUNK,
                num_idxs_reg=EMB_GCHUNK,
                elem_size=E,
            )
            nc.sync.dma_start(
                AP(emb_out.tensor, gi * EMB_GCHUNK * E,
                   [[E, 128], [128 * E, EMB_ROWS], [1, E]]),
                gbuf[:, :, :],
            )

        # ---- the scan (two chains in anti-phase) ----
        h_prev = [None, None]
        c_prev = [None, None]
        sig = {}        # (q, t) -> (gates-consumed tiles) for the DVE half
        chunk_info = {}  # chunk index -> (fc_ps, ohN, t0, nslot, remaining_fc)

        n_chunks = math.ceil(nstep / CH)
        ohT_tiles = {}

        def emit_chunk_prefetch(ci):
            t0 = ci * CH
            t1 = min(t0 + CH, nstep)
            nch = t1 - t0
            nslot = nch * NB
            xbc = oh_p.tile([V, nch * B], FP32, tag="xbc", name=f"xbc{ci}")
            nc.sync.dma_start(
                xbc[:, :],
                AP(xT.tensor, t0 * B, [[0, V], [B, nch], [1, B]]),
            )
            ohT = oh_p.tile([V, nch * B], FP16, tag="ohT", name=f"ohT{ci}")
            nc.vector.tensor_scalar(
                ohT[:, :], xbc[:, :], iota36[:, :1], None, ALU.is_equal)
            ohT_tiles[ci] = ohT
            ohN = oh_p.tile([128, nslot * V], FP32, tag="ohN", name=f"ohN{ci}")
            xb_ap = xB[:, :, :]
            nc.vector.tensor_tensor(
                ohN[:, :],
                AP(xb_ap.tensor, xb_ap.offset + t0 + 1,
                   [list(xb_ap.ap[0]), [1, nch], [T, NB], [0, V]]),
                _bc(iota_row[:, :], [(0, nch), (0, NB), (1, V)]),
                ALU.is_equal,
            )
            fc_ps = fc_pp.tile([128, nslot * V], FP32, tag="fcps",
                               name=f"fcps{ci}")
            chunk_info[ci] = [fc_ps, ohN, t0, nslot, 2 * nch]

        def emit_mm_act(q, t):
            ci = t // CH
            tr = t - ci * CH
            ohT = ohT_tiles[ci]
            gates = gates_pp.tile([128, G4], FP32, tag=f"gates{q}",
                                  name=f"g{q}_{t}")
            for m in range(8):
                r0, r1 = BLOCK_ROWS[m]
                nc.tensor.matmul(
                    gates[:, m * 128:(m + 1) * 128],
                    W2_sb[:, r0:r1],
                    ohT[:, tr * B + q * BC: tr * B + q * BC + BC],
                    start=(m % 4 == 0),
                    stop=(t == 0 and m % 4 == 3),
                )
            if t > 0:
                for m in range(8):
                    r0, r1 = BLOCK_ROWS[m]
                    for j in range(2):
                        nc.tensor.matmul(
                            gates[:, m * 128:(m + 1) * 128],
                            whh_sb[j][:, r0:r1],
                            h_prev[q][:, j * 128:(j + 1) * 128],
                            start=False,
                            stop=(m % 4 == 3 and j == 1),
                        )
            ifo = work_p.tile([128, 768], FP32, tag=f"ifo{q}",
                              name=f"ifo{q}_{t}")
            nc.scalar.activation(ifo[:, :], gates[:, 0:768], AF.Sigmoid)
            gg = work_p.tile([128, 256], FP32, tag=f"g{q}", name=f"gg{q}_{t}")
            nc.scalar.activation(gg[:, :], gates[:, 768:1024], AF.Tanh)
            sig[(q, t)] = (ifo, gg)

        def emit_dve(q, t):
            ci = t // CH
            tr = t - ci * CH
            info = chunk_info[ci]
            fc_ps, _, t0c, nslot, _ = info
            ifo, gg = sig.pop((q, t))
            ig = work_p.tile([128, 256], FP32, tag=f"ig{q}", name=f"ig{q}_{t}")
            nc.vector.tensor_tensor(ig[:, :], ifo[:, 0:256], gg[:, :], ALU.mult)
            if t == 0:
                c_new = ig
            else:
                fcs = work_p.tile([128, 256], FP32, tag=f"fc{q}",
                                  name=f"fcs{q}_{t}")
                nc.vector.tensor_tensor(
                    fcs[:, :], ifo[:, 256:512], c_prev[q][:, :], ALU.mult)
                c_new = state_p.tile([128, 256], FP32, tag=f"c{q}",
                                     name=f"c{q}_{t}")
                nc.vector.tensor_tensor(c_new[:, :], ig[:, :], fcs[:, :], ALU.add)
            c_prev[q] = c_new
            th = work_p.tile([128, 256], FP32, tag=f"th{q}", name=f"th{q}_{t}")
            nc.scalar.activation(th[:, :], c_new[:, :], AF.Tanh)
            h_new = state_p.tile([128, 256], FP16, tag=f"h{q}",
                                 name=f"h{q}_{t}")
            nc.vector.tensor_tensor(h_new[:, :], ifo[:, 512:768], th[:, :],
                                    ALU.mult)
            h_prev[q] = h_new
            slot = tr * NB + q
            for j in range(2):
                nc.tensor.matmul(
                    fc_ps[:, slot * V:(slot + 1) * V],
                    h_new[:, j * 128:(j + 1) * 128],
                    wfc_sb[j][:, :],
                    start=(slot == 0 and j == 0),
                    stop=(slot == nslot - 1 and j == 1),
                )
            info[4] -= 1
            if info[4] == 0:
                emit_chunk_epilogue(ci)

        def emit_chunk_epilogue(ci):
            fc_ps, ohN, t0, nslot, _ = chunk_info.pop(ci)
            biased = work_p.tile([128, nslot * V], FP32, tag="biased",
                                 name=f"biased{ci}")
            nc.vector.tensor_tensor(
                biased[:, :], fc_ps[:, :], bfc_tile[:, :nslot * V], ALU.add)
            nc.scalar.activation(
                relu_ring[:, t0 * NB * V:(t0 * NB + nslot) * V],
                biased[:, :], AF.Relu)
            selp = work_p.tile([128, nslot * V], FP32, tag="selp",
                               name=f"selp{ci}")
            rr_ap = relu_ring[:, :]
            nc.vector.tensor_tensor(
                selp[:, :],
                AP(rr_ap.tensor, rr_ap.offset + t0 * NB * V,
                   [list(rr_ap.ap[0]), [1, nslot * V]]),
                ohN[:, :], ALU.mult)
            nc.vector.tensor_reduce(
                out=sel_ring[:, t0 * NB:t0 * NB + nslot],
                in_=AP(selp[:, :].tensor, selp[:, :].offset,
                       [list(selp[:, :].ap[0]), [V, nslot], [1, V]]),
                axis=mybir.AxisListType.X,
                op=ALU.add,
            )

        emit_chunk_prefetch(0)
        for t in range(nstep):
            ci = t // CH
            if t % CH == 0 and ci + 1 < n_chunks:
                emit_chunk_prefetch(ci + 1)
            emit_mm_act(0, t)
            if t > 0:
                emit_dve(1, t - 1)
            emit_mm_act(1, t)
            emit_dve(0, t)
        emit_dve(1, nstep - 1)

        # ---- end phase: exp/ln (one ACT table switch), log-probs, NLL ----
        logZ = ring_p.tile([128, nstep * NB], FP32, tag="logZ")
        n_slots_total = nstep * NB
        piece_slots = 64
        off = 0
        while off < n_slots_total:
            ps = min(piece_slots, n_slots_total - off)
            expx = end_p.tile([128, piece_slots * V], FP32, tag="expx")
            nc.scalar.activation(
                expx[:, :ps * V],
                relu_ring[:, off * V:(off + ps) * V], AF.Exp)
            nc.vector.tensor_reduce(
                out=logZ[:, off:off + ps],
                in_=AP(expx[:, :].tensor, expx[:, :].offset,
                       [list(expx[:, :].ap[0]), [V, ps], [1, V]]),
                axis=mybir.AxisListType.X,
                op=ALU.add,
            )
            off += ps
        sumsZ = logZ
        logZ = ring_p.tile([128, nstep * NB], FP32, tag="logZ2")
        nc.scalar.activation(logZ[:, :], sumsZ[:, :], AF.Ln)

        # log_probs = relu_logits - logZ (broadcast over V); DMA out per piece
        off = 0
        while off < n_slots_total:
            ps = min(piece_slots, n_slots_total - off)
            lp_sb = end_p.tile([128, piece_slots * V], FP32, tag="lp")
            nc.vector.tensor_tensor(
                lp_sb[:, :ps * V],
                AP(relu_ring[:, :].tensor, relu_ring[:, :].offset + off * V,
                   [list(relu_ring[:, :].ap[0]), [V, ps], [1, V]]),
                AP(logZ[:, :].tensor, logZ[:, :].offset + off,
                   [list(logZ[:, :].ap[0]), [1, ps], [0, V]]),
                ALU.subtract,
            )
            # slots are (t*NB + q); pieces are multiples of NB
            assert off % NB == 0 and (ps % NB == 0 or off + ps == n_slots_total)
            tcnt = ps // NB
            tstart = off // NB
            for q in range(NB):
                nc.sync.dma_start(
                    AP(lp_out.tensor, tstart * V + q * BC * nstep * V,
                       [[nstep * V, 128], [V, tcnt], [1, V]]),
                    AP(lp_sb[:, :].tensor, lp_sb[:, :].offset + q * V,
                       [list(lp_sb[:, :].ap[0]), [NB * V, tcnt], [1, V]]),
                )
            off += ps

        # mask[p, (t, q)] = (x[q*128+p, t+1] != IGNORE)
        mask = ring_p.tile([128, nstep * NB], FP32, tag="mask")
        nc.vector.tensor_scalar(
            mask[:, :],
            AP(xB[:, :, :].tensor, xB[:, :, :].offset + 1,
               [list(xB[:, :, :].ap[0]), [1, nstep], [T, NB]]),
            float(IGNORE), None, ALU.not_equal)

        # nll contribution per slot: (logZ - sel) * mask, then reduce over t
        nll_slot = ring_p.tile([128, nstep * NB], FP32, tag="nll_slot")
        nc.vector.tensor_tensor(
            nll_slot[:, :], logZ[:, :], sel_ring[:, :], ALU.subtract)
        nc.vector.tensor_tensor(
            nll_slot[:, :], nll_slot[:, :], mask[:, :], ALU.mult)
        nllt = end_p.tile([128, NB], FP32, tag="nllt")
        nc.vector.tensor_reduce(
            out=nllt[:, :],
            in_=AP(nll_slot[:, :].tensor, nll_slot[:, :].offset,
                   [list(nll_slot[:, :].ap[0]), [1, NB], [NB, nstep]]),
            axis=mybir.AxisListType.X,
            op=ALU.add,
        )
        nc.sync.dma_start(nll_out[:, :], nllt[:, :])

    nc.compile()
    return nc


_PROGRAM = None


def _get_program():
    global _PROGRAM
    if _PROGRAM is None:
        _PROGRAM = build_program(NCORES)
    return _PROGRAM


def host_prep(x, emb_table, w_ih, w_hh, b_ih, b_fc, b_hh, w_fc):
    """Fold weights and build the per-core input maps."""
    x = np.asarray(x).astype(np.int32)
    emb_table = np.asarray(emb_table, dtype=np.float32)
    W2 = emb_table @ np.asarray(w_ih, np.float32).T \
        + np.asarray(b_ih, np.float32) + np.asarray(b_hh, np.float32)
    W2 = W2.astype(np.float16)
    whhT = np.ascontiguousarray(np.asarray(w_hh, np.float32).T.astype(np.float16))
    wfcT = np.ascontiguousarray(np.asarray(w_fc, np.float32).T.astype(np.float16))
    bfc = np.asarray(b_fc, np.float32)

    in_maps = []
    for c in range(NCORES):
        xs = x[c * B:(c + 1) * B]  # [256, 128]
        xf = xs.reshape(-1)
        idx16 = np.zeros((16, NTOK // 16), np.int16)
        ii = np.arange(NTOK)
        idx16[ii % 16, ii // 16] = xf.astype(np.int16)
        idx16 = np.tile(idx16, (8, 1))
        in_maps.append({
            "xidx16": idx16,
            "xflat32": np.ascontiguousarray(xf.astype(np.float32)),
            "xT": np.ascontiguousarray(xs.T.astype(np.float32)),
            "whhT": whhT,
            "W2": np.ascontiguousarray(W2),
            "wfcT": wfcT,
            "bfc": bfc,
            "etab": emb_table,
        })
    return x, in_maps


def assemble(x, results):
    """Gather per-core outputs into the full-shape reference outputs."""
    lp = np.concatenate([r["lp_out"] for r in results], axis=0)
    emb = np.concatenate(
        [r["emb_out"].reshape(B, T, E) for r in results], axis=0)
    nll = np.stack([r["nll_out"] for r in results])  # [NC, 128, NB]
    # nll[c, p, q] is the masked NLL sum of batch element c*B + q*128 + p
    nll_sum = nll.transpose(0, 2, 1).reshape(B_FULL)
    lengths = (x != IGNORE).sum(axis=1).astype(np.float32)
    sample_loss = nll_sum / lengths
    mean_loss = np.float32(sample_loss.mean())
    return lp, emb, sample_loss, mean_loss


def kernel(x, emb_table, w_ih, w_hh, b_ih, b_fc, b_hh, w_fc):
    from concourse.bass_utils import run_bass_kernel_spmd

    x, in_maps = host_prep(x, emb_table, w_ih, w_hh, b_ih, b_fc, b_hh, w_fc)
    nc = _get_program()
    res = run_bass_kernel_spmd(nc, in_maps, core_ids=list(range(NCORES)))
    return assemble(x, res.results)


# revision 19
# speedup vs baseline: 2.5763x; 1.0058x over previous
"""Trainium2 Bass kernel for the AutoregressiveLSTM problem.

Data-parallel over 8 NeuronCores: batch 2048 -> 256 per core.

Per-core layout ("feature-major"): the LSTM state h and cell c live as
hT [H, B] packed into SBUF tiles [128, 2*128] (H-chunk-major), so the
recurrent matmul gatesT[4H, B] = w_hh @ h needs no per-step transposes
and the FC layer can use hT chunks directly as the stationary operand.

The input-side gate contribution xg[t] = W2[x[:, t]] (W2 = emb_table @
w_ih.T + b_ih + b_hh, folded on host) is accumulated into the same PSUM
banks via one-hot matmuls with K=36.

exp/ln (log-softmax) are deferred to a single end phase because Sigmoid/
Tanh/Relu share one ACT table set while Exp/Ln live in another (~2.7us
per table switch if interleaved).

The emb output is a pure gather -> indirect DMA, no compute engines.
"""

import math
import os
from contextlib import ExitStack

import numpy as np

import concourse.bass as bass
import concourse.tile as tile
from concourse import bacc, mybir
from concourse.bass import AP

FP32 = mybir.dt.float32
FP16 = mybir.dt.float16
I32 = mybir.dt.int32
AF = mybir.ActivationFunctionType
ALU = mybir.AluOpType

# Problem constants (hardcoded per contract)
B_FULL, T, V, E, H = 2048, 128, 36, 128, 256
NCORES = 8
B = B_FULL // NCORES          # 256 per core
NB = 2                        # batch chains per core
BC = B // NB                  # 128 per chain
G4 = 4 * H                    # 1024
NSTEP = T - 1                 # 127 LSTM steps
CH = 7                        # steps per chunk (14 FC slots of 36 = 504 <= 512 psum bank)
NTOK = B * T                  # 32768 tokens per core (for emb gather)
IGNORE = 35

# gatesT PSUM col-block m covers gate rows R[m] (PyTorch gate order i,f,g,o
# in the 4H dim). Block packing: bank0 = [i0 i1 f0 f1], bank1 = [o0 o1 g0 g1]
# so sigmoid covers cols 0:768 and tanh covers cols 768:1024.
BLOCK_ROWS = [
    (0, 128), (128, 256),          # i
    (256, 384), (384, 512),        # f
    (768, 896), (896, 1024),       # o
    (512, 640), (640, 768),        # g
]

EMB_GCHUNK = 1024              # tokens per dma_gather call (desc-ring limit)
N_EMB_G = NTOK // EMB_GCHUNK   # 32
EMB_ROWS = EMB_GCHUNK // 128   # 8 gathered rows per out free slot


def _bc(ap: AP, dims) -> AP:
    """Build a broadcast/strided view of an AP: dims = [(step, count), ...]
    appended after the partition dim. step refers to the flat free offset of
    the underlying tile."""
    part = ap.ap[0]
    return AP(ap.tensor, ap.offset, [list(part)] + [[s, c] for s, c in dims])


def build_program(num_devices: int = NCORES, nstep: int = NSTEP):
    nc = bacc.Bacc(
        "TRN2",
        target_bir_lowering=False,
        debug=False,
        enable_asserts=False,
        num_devices=num_devices,
    )

    # ---- DRAM tensors ----
    xidx16 = nc.dram_tensor("xidx16", [128, NTOK // 16], mybir.dt.int16,
                            kind="ExternalInput").ap()
    xflat32 = nc.dram_tensor("xflat32", [NTOK], FP32, kind="ExternalInput").ap()
    xT = nc.dram_tensor("xT", [T, B], FP32, kind="ExternalInput").ap()
    whhT_d = nc.dram_tensor("whhT", [H, G4], FP16, kind="ExternalInput").ap()
    W2_d = nc.dram_tensor("W2", [V, G4], FP16, kind="ExternalInput").ap()
    wfcT_d = nc.dram_tensor("wfcT", [H, V], FP16, kind="ExternalInput").ap()
    bfc_d = nc.dram_tensor("bfc", [V], FP32, kind="ExternalInput").ap()
    etab_d = nc.dram_tensor("etab", [V, E], FP32, kind="ExternalInput").ap()

    emb_out = nc.dram_tensor("emb_out", [NTOK, E], FP32, kind="ExternalOutput").ap()
    lp_out = nc.dram_tensor("lp_out", [B, nstep, V], FP32, kind="ExternalOutput").ap()
    nll_out = nc.dram_tensor("nll_out", [128, NB], FP32, kind="ExternalOutput").ap()

    with tile.TileContext(nc) as tc, ExitStack() as ctx:
        const_p = ctx.enter_context(tc.tile_pool(name="const", bufs=1))
        ring_p = ctx.enter_context(tc.tile_pool(name="ring", bufs=1))
        state_p = ctx.enter_context(tc.tile_pool(name="state", bufs=2))
        work_p = ctx.enter_context(tc.tile_pool(name="work", bufs=2))
        oh_p = ctx.enter_context(tc.tile_pool(name="oh", bufs=2))
        emb_p = ctx.enter_context(tc.tile_pool(name="emb", bufs=2))
        end_p = ctx.enter_context(tc.tile_pool(name="end", bufs=2))
        gates_pp0 = ctx.enter_context(tc.tile_pool(name="gates0", bufs=2, space="PSUM"))
        gates_pp1 = ctx.enter_context(tc.tile_pool(name="gates1", bufs=1, space="PSUM"))
        fc_pp = ctx.enter_context(tc.tile_pool(name="fc", bufs=2, space="PSUM"))

        # ---- static loads ----
        whh_sb = []
        for j in range(2):
            wt = const_p.tile([128, G4], FP16, tag=f"whh{j}")
            nc.sync.dma_start(wt[:, :], whhT_d[j * 128:(j + 1) * 128, :])
            whh_sb.append(wt)
        W2_sb = const_p.tile([V, G4], FP16, tag="w2")
        nc.sync.dma_start(W2_sb[:, :], W2_d[:, :])
        wfc_sb = []
        for j in range(2):
            wt = const_p.tile([128, V], FP16, tag=f"wfc{j}")
            nc.sync.dma_start(wt[:, :], wfcT_d[j * 128:(j + 1) * 128, :])
            wfc_sb.append(wt)
        etab_sb = const_p.tile([V, E], FP32, tag="etab")
        nc.sync.dma_start(etab_sb[:, :], etab_d[:, :])

        bfc_tile = const_p.tile([128, 2 * CH * V], FP32, tag="bfc")
        nc.sync.dma_start(
            bfc_tile[:, :],
            AP(bfc_d.tensor, 0, [[0, 128], [0, 2 * CH], [1, V]]),
        )

        iota36 = const_p.tile([V, 1], FP32, tag="iota36")
        nc.gpsimd.iota(iota36[:, :], pattern=[[0, 1]], base=0,
                       channel_multiplier=1,
                       allow_small_or_imprecise_dtypes=True)
        iota_row = const_p.tile([128, V], FP32, tag="iotar")
        nc.gpsimd.iota(iota_row[:, :], pattern=[[1, V]], base=0,
                       channel_multiplier=0,
                       allow_small_or_imprecise_dtypes=True)

        # x in batch-chain layout: xB[p, q, t] = x[q*128+p, t]
        xB = const_p.tile([128, NB, T], FP32, tag="xB")
        nc.sync.dma_start(
            xB[:, :, :],
            AP(xflat32.tensor, 0, [[T, 128], [128 * T, NB], [1, T]]),
        )

        # rings kept for the whole scan
        relu_ring = ring_p.tile([128, nstep * NB * V], FP32, tag="relu_ring")
        sel_ring = ring_p.tile([128, nstep * NB], FP32, tag="sel_ring")

        # ---- emb output via dma_gather (independent of the scan) ----
        idx16_sb = const_p.tile([128, NTOK // 16], mybir.dt.int16, tag="idx16")
        nc.sync.dma_start(idx16_sb[:, :], xidx16[:, :])
        npc = EMB_GCHUNK // 16     # idx columns per gather chunk
        for gi in range(N_EMB_G):
            gbuf = emb_p.tile([128, EMB_ROWS, E], FP32, tag="gbuf")
            nc.gpsimd.dma_gather(
                out_ap=gbuf[:, :, :],
                in_ap=etab_d[:, :],
                idxs_ap=idx16_sb[:, gi * npc:(gi + 1) * npc],
                num_idxs=EMB_GCHUNK,
                num_idxs_reg=EMB_GCHUNK,
                elem_size=E,
            )
            nc.sync.dma_start(
                AP(emb_out.tensor, gi * EMB_GCHUNK * E,
                   [[E, 128], [128 * E, EMB_ROWS], [1, E]]),
                gbuf[:, :, :],
            )

        # ---- the scan (two chains in anti-phase) ----
        h_prev = [None, None]
        c_prev = [None, None]
        sig = {}        # (q, t) -> (gates-consumed tiles) for the DVE half
        chunk_info = {}  # chunk index -> (fc_ps, ohN, t0, nslot, remaining_fc)

        n_chunks = math.ceil(nstep / CH)
        ohT_tiles = {}

        def emit_chunk_prefetch(ci):
            t0 = ci * CH
            t1 = min(t0 + CH, nstep)
            nch = t1 - t0
            nslot = nch * NB
            xbc = oh_p.tile([V, nch * B], FP32, tag="xbc", name=f"xbc{ci}")
            nc.sync.dma_start(
                xbc[:, :],
                AP(xT.tensor, t0 * B, [[0, V], [B, nch], [1, B]]),
            )
            ohT = oh_p.tile([V, nch * B], FP16, tag="ohT", name=f"ohT{ci}")
            nc.vector.tensor_scalar(
                ohT[:, :], xbc[:, :], iota36[:, :1], None, ALU.is_equal)
            ohT_tiles[ci] = ohT
            ohN = oh_p.tile([128, nslot * V], FP32, tag="ohN", name=f"ohN{ci}")
            xb_ap = xB[:, :, :]
            nc.vector.tensor_tensor(
                ohN[:, :],
                AP(xb_ap.tensor, xb_ap.offset + t0 + 1,
                   [list(xb_ap.ap[0]), [1, nch], [T, NB], [0, V]]),
                _bc(iota_row[:, :], [(0, nch), (0, NB), (1, V)]),
                ALU.is_equal,
            )
            fc_ps = fc_pp.tile([128, nslot * V], FP32, tag="fcps",
                               name=f"fcps{ci}")
            chunk_info[ci] = [fc_ps, ohN, t0, nslot, 2 * nch]

        gates_tiles = {}

        def emit_oh(q, t):
            if t >= nstep:
                return
            ci = t // CH
            tr = t - ci * CH
            ohT = ohT_tiles[ci]
            pool = gates_pp0 if q == 0 else gates_pp1
            gates = pool.tile([128, G4], FP32, tag=f"gates{q}",
                              name=f"g{q}_{t}")
            for m in range(8):
                r0, r1 = BLOCK_ROWS[m]
                nc.tensor.matmul(
                    gates[:, m * 128:(m + 1) * 128],
                    W2_sb[:, r0:r1],
                    ohT[:, tr * B + q * BC: tr * B + q * BC + BC],
                    start=(m % 4 == 0),
                    stop=(t == 0 and m % 4 == 3),
                )
            gates_tiles[(q, t)] = gates

        def emit_rec_act(q, t):
            gates = gates_tiles.pop((q, t))
            if t > 0:
                for m in range(8):
                    r0, r1 = BLOCK_ROWS[m]
                    for j in range(2):
                        nc.tensor.matmul(
                            gates[:, m * 128:(m + 1) * 128],
                            whh_sb[j][:, r0:r1],
                            h_prev[q][:, j * 128:(j + 1) * 128],
                            start=False,
                            stop=(m % 4 == 3 and j == 1),
                        )
            ifo = work_p.tile([128, 768], FP32, tag=f"ifo{q}",
                              name=f"ifo{q}_{t}")
            nc.scalar.activation(ifo[:, :], gates[:, 0:768], AF.Sigmoid)
            gg = work_p.tile([128, 256], FP32, tag=f"g{q}", name=f"gg{q}_{t}")
            nc.scalar.activation(gg[:, :], gates[:, 768:1024], AF.Tanh)
            sig[(q, t)] = (ifo, gg)

        def emit_dve(q, t):
            ci = t // CH
            tr = t - ci * CH
            info = chunk_info[ci]
            fc_ps, _, t0c, nslot, _ = info
            ifo, gg = sig.pop((q, t))
            if t > 0:
                fcs = work_p.tile([128, 256], FP32, tag=f"fc{q}",
                                  name=f"fcs{q}_{t}")
                nc.vector.tensor_tensor(
                    fcs[:, :], ifo[:, 256:512], c_prev[q][:, :], ALU.mult)
            ig = work_p.tile([128, 256], FP32, tag=f"ig{q}", name=f"ig{q}_{t}")
            nc.vector.tensor_tensor(ig[:, :], ifo[:, 0:256], gg[:, :], ALU.mult)
            if t == 0:
                c_new = ig
            else:
                c_new = state_p.tile([128, 256], FP32, tag=f"c{q}",
                                     name=f"c{q}_{t}")
                nc.vector.tensor_tensor(c_new[:, :], ig[:, :], fcs[:, :], ALU.add)
            c_prev[q] = c_new
            th = work_p.tile([128, 256], FP32, tag=f"th{q}", name=f"th{q}_{t}")
            nc.scalar.activation(th[:, :], c_new[:, :], AF.Tanh)
            h_new = state_p.tile([128, 256], FP16, tag=f"h{q}",
                                 name=f"h{q}_{t}")
            nc.vector.tensor_tensor(h_new[:, :], ifo[:, 512:768], th[:, :],
                                    ALU.mult)
            h_prev[q] = h_new
            slot = tr * NB + q
            for j in range(2):
                nc.tensor.matmul(
                    fc_ps[:, slot * V:(slot + 1) * V],
                    h_new[:, j * 128:(j + 1) * 128],
                    wfc_sb[j][:, :],
                    start=(slot == 0 and j == 0),
                    stop=(slot == nslot - 1 and j == 1),
                )
            info[4] -= 1
            if info[4] == 0:
                emit_chunk_epilogue(ci)

        def emit_chunk_epilogue(ci):
            fc_ps, ohN, t0, nslot, _ = chunk_info.pop(ci)
            biased = work_p.tile([128, nslot * V], FP32, tag="biased",
                                 name=f"biased{ci}")
            nc.vector.tensor_tensor(
                biased[:, :], fc_ps[:, :], bfc_tile[:, :nslot * V], ALU.add)
            nc.scalar.activation(
                relu_ring[:, t0 * NB * V:(t0 * NB + nslot) * V],
                biased[:, :], AF.Relu)
            selp = work_p.tile([128, nslot * V], FP32, tag="selp",
                               name=f"selp{ci}")
            rr_ap = relu_ring[:, :]
            nc.vector.tensor_tensor(
                selp[:, :],
                AP(rr_ap.tensor, rr_ap.offset + t0 * NB * V,
                   [list(rr_ap.ap[0]), [1, nslot * V]]),
                ohN[:, :], ALU.mult)
            nc.vector.tensor_reduce(
                out=sel_ring[:, t0 * NB:t0 * NB + nslot],
                in_=AP(selp[:, :].tensor, selp[:, :].offset,
                       [list(selp[:, :].ap[0]), [V, nslot], [1, V]]),
                axis=mybir.AxisListType.X,
                op=ALU.add,
            )

        emit_chunk_prefetch(0)
        emit_oh(0, 0)
        emit_oh(1, 0)
        for t in range(nstep):
            ci = t // CH
            if t % CH == 0 and ci + 1 < n_chunks:
                emit_chunk_prefetch(ci + 1)
            emit_rec_act(0, t)
            emit_oh(0, t + 1)
            if t > 0:
                emit_dve(1, t - 1)
            emit_rec_act(1, t)
            emit_oh(1, t + 1)
            emit_dve(0, t)
        emit_dve(1, nstep - 1)

        # ---- end phase: exp/ln (one ACT table switch), log-probs, NLL ----
        logZ = ring_p.tile([128, nstep * NB], FP32, tag="logZ")
        n_slots_total = nstep * NB
        piece_slots = 64
        off = 0
        while off < n_slots_total:
            ps = min(piece_slots, n_slots_total - off)
            expx = end_p.tile([128, piece_slots * V], FP32, tag="expx")
            nc.scalar.activation(
                expx[:, :ps * V],
                relu_ring[:, off * V:(off + ps) * V], AF.Exp)
            nc.vector.tensor_reduce(
                out=logZ[:, off:off + ps],
                in_=AP(expx[:, :].tensor, expx[:, :].offset,
                       [list(expx[:, :].ap[0]), [V, ps], [1, V]]),
                axis=mybir.AxisListType.X,
                op=ALU.add,
            )
            off += ps
        sumsZ = logZ
        logZ = ring_p.tile([128, nstep * NB], FP32, tag="logZ2")
        nc.scalar.activation(logZ[:, :], sumsZ[:, :], AF.Ln)

        # log_probs = relu_logits - logZ (broadcast over V); DMA out per piece
        off = 0
        while off < n_slots_total:
            ps = min(piece_slots, n_slots_total - off)
            lp_sb = end_p.tile([128, piece_slots * V], FP32, tag="lp")
            nc.vector.tensor_tensor(
                lp_sb[:, :ps * V],
                AP(relu_ring[:, :].tensor, relu_ring[:, :].offset + off * V,
                   [list(relu_ring[:, :].ap[0]), [V, ps], [1, V]]),
                AP(logZ[:, :].tensor, logZ[:, :].offset + off,
                   [list(logZ[:, :].ap[0]), [1, ps], [0, V]]),
                ALU.subtract,
            )
            # slots are (t*NB + q); pieces are multiples of NB
            assert off % NB == 0 and (ps % NB == 0 or off + ps == n_slots_total)
            tcnt = ps // NB
            tstart = off // NB
            for q in range(NB):
                nc.sync.dma_start(
                    AP(lp_out.tensor, tstart * V + q * BC * nstep * V,
                       [[nstep * V, 128], [V, tcnt], [1, V]]),
                    AP(lp_sb[:, :].tensor, lp_sb[:, :].offset + q * V,
                       [list(lp_sb[:, :].ap[0]), [NB * V, tcnt], [1, V]]),
                )
            off += ps

        # mask[p, (t, q)] = (x[q*128+p, t+1] != IGNORE)
        mask = ring_p.tile([128, nstep * NB], FP32, tag="mask")
        nc.vector.tensor_scalar(
            mask[:, :],
            AP(xB[:, :, :].tensor, xB[:, :, :].offset + 1,
               [list(xB[:, :, :].ap[0]), [1, nstep], [T, NB]]),
            float(IGNORE), None, ALU.not_equal)

        # nll contribution per slot: (logZ - sel) * mask, then reduce over t
        nll_slot = ring_p.tile([128, nstep * NB], FP32, tag="nll_slot")
        nc.vector.tensor_tensor(
            nll_slot[:, :], logZ[:, :], sel_ring[:, :], ALU.subtract)
        nc.vector.tensor_tensor(
            nll_slot[:, :], nll_slot[:, :], mask[:, :], ALU.mult)
        nllt = end_p.tile([128, NB], FP32, tag="nllt")
        nc.vector.tensor_reduce(
            out=nllt[:, :],
            in_=AP(nll_slot[:, :].tensor, nll_slot[:, :].offset,
                   [list(nll_slot[:, :].ap[0]), [1, NB], [NB, nstep]]),
            axis=mybir.AxisListType.X,
            op=ALU.add,
        )
        nc.sync.dma_start(nll_out[:, :], nllt[:, :])

    nc.compile()
    return nc


_PROGRAM = None


def _get_program():
    global _PROGRAM
    if _PROGRAM is None:
        _PROGRAM = build_program(NCORES)
    return _PROGRAM


def host_prep(x, emb_table, w_ih, w_hh, b_ih, b_fc, b_hh, w_fc):
    """Fold weights and build the per-core input maps."""
    x = np.asarray(x).astype(np.int32)
    emb_table = np.asarray(emb_table, dtype=np.float32)
    W2 = emb_table @ np.asarray(w_ih, np.float32).T \
        + np.asarray(b_ih, np.float32) + np.asarray(b_hh, np.float32)
    W2 = W2.astype(np.float16)
    whhT = np.ascontiguousarray(np.asarray(w_hh, np.float32).T.astype(np.float16))
    wfcT = np.ascontiguousarray(np.asarray(w_fc, np.float32).T.astype(np.float16))
    bfc = np.asarray(b_fc, np.float32)

    in_maps = []
    for c in range(NCORES):
        xs = x[c * B:(c + 1) * B]  # [256, 128]
        xf = xs.reshape(-1)
        idx16 = np.zeros((16, NTOK // 16), np.int16)
        ii = np.arange(NTOK)
        idx16[ii % 16, ii // 16] = xf.astype(np.int16)
        idx16 = np.tile(idx16, (8, 1))
        in_maps.append({
            "xidx16": idx16,
            "xflat32": np.ascontiguousarray(xf.astype(np.float32)),
            "xT": np.ascontiguousarray(xs.T.astype(np.float32)),
            "whhT": whhT,
            "W2": np.ascontiguousarray(W2),
            "wfcT": wfcT,
            "bfc": bfc,
            "etab": emb_table,
        })
    return x, in_maps


def assemble(x, results):
    """Gather per-core outputs into the full-shape reference outputs."""
    lp = np.concatenate([r["lp_out"] for r in results], axis=0)
    emb = np.concatenate(
        [r["emb_out"].reshape(B, T, E) for r in results], axis=0)
    nll = np.stack([r["nll_out"] for r in results])  # [NC, 128, NB]
    # nll[c, p, q] is the masked NLL sum of batch element c*B + q*128 + p
    nll_sum = nll.transpose(0, 2, 1).reshape(B_FULL)
    lengths = (x != IGNORE).sum(axis=1).astype(np.float32)
    sample_loss = nll_sum / lengths
    mean_loss = np.float32(sample_loss.mean())
    return lp, emb, sample_loss, mean_loss


def kernel(x, emb_table, w_ih, w_hh, b_ih, b_fc, b_hh, w_fc):
    from concourse.bass_utils import run_bass_kernel_spmd

    x, in_maps = host_prep(x, emb_table, w_ih, w_hh, b_ih, b_fc, b_hh, w_fc)
    nc = _get_program()
    res = run_bass_kernel_spmd(nc, in_maps, core_ids=list(range(NCORES)))
    return assemble(x, res.results)


# revision 20
# speedup vs baseline: 2.6111x; 1.0135x over previous
"""Trainium2 Bass kernel for the AutoregressiveLSTM problem.

Data-parallel over 8 NeuronCores: batch 2048 -> 256 per core.

Per-core layout ("feature-major"): the LSTM state h and cell c live as
hT [H, B] packed into SBUF tiles [128, 2*128] (H-chunk-major), so the
recurrent matmul gatesT[4H, B] = w_hh @ h needs no per-step transposes
and the FC layer can use hT chunks directly as the stationary operand.

The input-side gate contribution xg[t] = W2[x[:, t]] (W2 = emb_table @
w_ih.T + b_ih + b_hh, folded on host) is accumulated into the same PSUM
banks via one-hot matmuls with K=36.

exp/ln (log-softmax) are deferred to a single end phase because Sigmoid/
Tanh/Relu share one ACT table set while Exp/Ln live in another (~2.7us
per table switch if interleaved).

The emb output is a pure gather -> indirect DMA, no compute engines.
"""

import math
import os
from contextlib import ExitStack

import numpy as np

import concourse.bass as bass
import concourse.tile as tile
from concourse import bacc, mybir
from concourse.bass import AP

FP32 = mybir.dt.float32
FP16 = mybir.dt.float16
I32 = mybir.dt.int32
AF = mybir.ActivationFunctionType
ALU = mybir.AluOpType

# Problem constants (hardcoded per contract)
B_FULL, T, V, E, H = 2048, 128, 36, 128, 256
NCORES = 8
B = B_FULL // NCORES          # 256 per core
NB = 2                        # batch chains per core
BC = B // NB                  # 128 per chain
G4 = 4 * H                    # 1024
NSTEP = T - 1                 # 127 LSTM steps
CH = 7                        # steps per chunk (14 FC slots of 36 = 504 <= 512 psum bank)
NTOK = B * T                  # 32768 tokens per core (for emb gather)
IGNORE = 35

# gatesT PSUM col-block m covers gate rows R[m] (PyTorch gate order i,f,g,o
# in the 4H dim). Block packing: bank0 = [i0 i1 f0 f1], bank1 = [o0 o1 g0 g1]
# so sigmoid covers cols 0:768 and tanh covers cols 768:1024.
BLOCK_ROWS = [
    (0, 128), (128, 256),          # i
    (256, 384), (384, 512),        # f
    (768, 896), (896, 1024),       # o
    (512, 640), (640, 768),        # g
]

EMB_GCHUNK = 1024              # tokens per dma_gather call (desc-ring limit)
N_EMB_G = NTOK // EMB_GCHUNK   # 32
EMB_ROWS = EMB_GCHUNK // 128   # 8 gathered rows per out free slot


def _bc(ap: AP, dims) -> AP:
    """Build a broadcast/strided view of an AP: dims = [(step, count), ...]
    appended after the partition dim. step refers to the flat free offset of
    the underlying tile."""
    part = ap.ap[0]
    return AP(ap.tensor, ap.offset, [list(part)] + [[s, c] for s, c in dims])


def build_program(num_devices: int = NCORES, nstep: int = NSTEP):
    nc = bacc.Bacc(
        "TRN2",
        target_bir_lowering=False,
        debug=False,
        enable_asserts=False,
        num_devices=num_devices,
    )

    # ---- DRAM tensors ----
    xidx16 = nc.dram_tensor("xidx16", [128, NTOK // 16], mybir.dt.int16,
                            kind="ExternalInput").ap()
    xflat32 = nc.dram_tensor("xflat32", [NTOK], FP32, kind="ExternalInput").ap()
    xT = nc.dram_tensor("xT", [T, B], FP32, kind="ExternalInput").ap()
    whhT_d = nc.dram_tensor("whhT", [H, G4], FP16, kind="ExternalInput").ap()
    W2_d = nc.dram_tensor("W2", [V, G4], FP16, kind="ExternalInput").ap()
    wfcT_d = nc.dram_tensor("wfcT", [H, V], FP16, kind="ExternalInput").ap()
    bfc_d = nc.dram_tensor("bfc", [V], FP32, kind="ExternalInput").ap()
    etab_d = nc.dram_tensor("etab", [V, E], FP32, kind="ExternalInput").ap()

    emb_out = nc.dram_tensor("emb_out", [NTOK, E], FP32, kind="ExternalOutput").ap()
    lp_out = nc.dram_tensor("lp_out", [B, nstep, V], FP32, kind="ExternalOutput").ap()
    nll_out = nc.dram_tensor("nll_out", [128, NB], FP32, kind="ExternalOutput").ap()

    with tile.TileContext(nc) as tc, ExitStack() as ctx:
        const_p = ctx.enter_context(tc.tile_pool(name="const", bufs=1))
        ring_p = ctx.enter_context(tc.tile_pool(name="ring", bufs=1))
        state_p = ctx.enter_context(tc.tile_pool(name="state", bufs=2))
        work_p = ctx.enter_context(tc.tile_pool(name="work", bufs=2))
        oh_p = ctx.enter_context(tc.tile_pool(name="oh", bufs=2))
        emb_p = ctx.enter_context(tc.tile_pool(name="emb", bufs=2))
        end_p = ctx.enter_context(tc.tile_pool(name="end", bufs=2))
        gates_pp0 = ctx.enter_context(tc.tile_pool(name="gates0", bufs=2, space="PSUM"))
        gates_pp1 = ctx.enter_context(tc.tile_pool(name="gates1", bufs=1, space="PSUM"))
        fc_pp = ctx.enter_context(tc.tile_pool(name="fc", bufs=2, space="PSUM"))

        # ---- static loads ----
        whh_sb = []
        for j in range(2):
            wt = const_p.tile([128, G4], FP16, tag=f"whh{j}")
            nc.sync.dma_start(wt[:, :], whhT_d[j * 128:(j + 1) * 128, :])
            whh_sb.append(wt)
        W2_sb = const_p.tile([V, G4], FP16, tag="w2")
        nc.sync.dma_start(W2_sb[:, :], W2_d[:, :])
        wfc_sb = []
        for j in range(2):
            wt = const_p.tile([128, V], FP16, tag=f"wfc{j}")
            nc.sync.dma_start(wt[:, :], wfcT_d[j * 128:(j + 1) * 128, :])
            wfc_sb.append(wt)
        etab_sb = const_p.tile([V, E], FP32, tag="etab")
        nc.sync.dma_start(etab_sb[:, :], etab_d[:, :])

        bfc_tile = const_p.tile([128, 2 * CH * V], FP32, tag="bfc")
        nc.sync.dma_start(
            bfc_tile[:, :],
            AP(bfc_d.tensor, 0, [[0, 128], [0, 2 * CH], [1, V]]),
        )

        iota36 = const_p.tile([V, 1], FP32, tag="iota36")
        nc.gpsimd.iota(iota36[:, :], pattern=[[0, 1]], base=0,
                       channel_multiplier=1,
                       allow_small_or_imprecise_dtypes=True)
        iota_row = const_p.tile([128, V], FP32, tag="iotar")
        nc.gpsimd.iota(iota_row[:, :], pattern=[[1, V]], base=0,
                       channel_multiplier=0,
                       allow_small_or_imprecise_dtypes=True)

        # x in batch-chain layout: xB[p, q, t] = x[q*128+p, t]
        xB = const_p.tile([128, NB, T], FP32, tag="xB")
        nc.sync.dma_start(
            xB[:, :, :],
            AP(xflat32.tensor, 0, [[T, 128], [128 * T, NB], [1, T]]),
        )

        # rings kept for the whole scan
        relu_ring = ring_p.tile([128, nstep * NB * V], FP32, tag="relu_ring")
        sel_ring = ring_p.tile([128, nstep * NB], FP32, tag="sel_ring")

        # ---- emb output via dma_gather (independent of the scan) ----
        idx16_sb = const_p.tile([128, NTOK // 16], mybir.dt.int16, tag="idx16")
        nc.sync.dma_start(idx16_sb[:, :], xidx16[:, :])
        npc = EMB_GCHUNK // 16     # idx columns per gather chunk
        for gi in range(N_EMB_G):
            gbuf = emb_p.tile([128, EMB_ROWS, E], FP32, tag="gbuf")
            nc.gpsimd.dma_gather(
                out_ap=gbuf[:, :, :],
                in_ap=etab_d[:, :],
                idxs_ap=idx16_sb[:, gi * npc:(gi + 1) * npc],
                num_idxs=EMB_GCHUNK,
                num_idxs_reg=EMB_GCHUNK,
                elem_size=E,
            )
            nc.sync.dma_start(
                AP(emb_out.tensor, gi * EMB_GCHUNK * E,
                   [[E, 128], [128 * E, EMB_ROWS], [1, E]]),
                gbuf[:, :, :],
            )

        # ---- the scan (two chains in anti-phase) ----
        h_prev = [None, None]
        c_prev = [None, None]
        sig = {}        # (q, t) -> (gates-consumed tiles) for the DVE half
        chunk_info = {}  # chunk index -> (fc_ps, ohN, t0, nslot, remaining_fc)

        n_chunks = math.ceil(nstep / CH)
        ohT_tiles = {}

        def emit_chunk_prefetch(ci):
            t0 = ci * CH
            t1 = min(t0 + CH, nstep)
            nch = t1 - t0
            nslot = nch * NB
            xbc = oh_p.tile([V, nch * B], FP32, tag="xbc", name=f"xbc{ci}")
            nc.sync.dma_start(
                xbc[:, :],
                AP(xT.tensor, t0 * B, [[0, V], [B, nch], [1, B]]),
            )
            ohT = oh_p.tile([V, nch * B], FP16, tag="ohT", name=f"ohT{ci}")
            nc.vector.tensor_scalar(
                ohT[:, :], xbc[:, :], iota36[:, :1], None, ALU.is_equal)
            ohT_tiles[ci] = ohT
            ohN = oh_p.tile([128, nslot * V], FP32, tag="ohN", name=f"ohN{ci}")
            xb_ap = xB[:, :, :]
            nc.vector.tensor_tensor(
                ohN[:, :],
                AP(xb_ap.tensor, xb_ap.offset + t0 + 1,
                   [list(xb_ap.ap[0]), [1, nch], [T, NB], [0, V]]),
                _bc(iota_row[:, :], [(0, nch), (0, NB), (1, V)]),
                ALU.is_equal,
            )
            fc_ps = fc_pp.tile([128, nslot * V], FP32, tag="fcps",
                               name=f"fcps{ci}")
            chunk_info[ci] = [fc_ps, ohN, t0, nslot, 2 * nch]

        gates_tiles = {}

        def emit_oh(q, t):
            if t >= nstep:
                return
            ci = t // CH
            tr = t - ci * CH
            ohT = ohT_tiles[ci]
            pool = gates_pp0 if q == 0 else gates_pp1
            gates = pool.tile([128, G4], FP32, tag=f"gates{q}",
                              name=f"g{q}_{t}")
            for m in range(8):
                r0, r1 = BLOCK_ROWS[m]
                nc.tensor.matmul(
                    gates[:, m * 128:(m + 1) * 128],
                    W2_sb[:, r0:r1],
                    ohT[:, tr * B + q * BC: tr * B + q * BC + BC],
                    start=(m % 4 == 0),
                    stop=(t == 0 and m % 4 == 3),
                )
            gates_tiles[(q, t)] = gates

        def emit_rec_act(q, t):
            gates = gates_tiles.pop((q, t))
            if t > 0:
                for m in range(8):
                    r0, r1 = BLOCK_ROWS[m]
                    for j in range(2):
                        nc.tensor.matmul(
                            gates[:, m * 128:(m + 1) * 128],
                            whh_sb[j][:, r0:r1],
                            h_prev[q][:, j * 128:(j + 1) * 128],
                            start=False,
                            stop=(m % 4 == 3 and j == 1),
                        )
            # g-gate weights are pre-scaled x2 on host: tanh(x) = 2*sig(2x)-1,
            # so one sigmoid covers all four gates; the affine fix-up is
            # folded into the DVE ops below.
            ifog = work_p.tile([128, 1024], FP32, tag=f"ifo{q}",
                               name=f"ifo{q}_{t}")
            nc.scalar.activation(ifog[:, :], gates[:, :], AF.Sigmoid)
            sig[(q, t)] = ifog

        def emit_dve(q, t):
            ci = t // CH
            tr = t - ci * CH
            info = chunk_info[ci]
            fc_ps, _, t0c, nslot, _ = info
            ifog = sig.pop((q, t))
            if t > 0:
                fcs = work_p.tile([128, 256], FP32, tag=f"fc{q}",
                                  name=f"fcs{q}_{t}")
                nc.vector.tensor_tensor(
                    fcs[:, :], ifog[:, 256:512], c_prev[q][:, :], ALU.mult)
            # X = (sig(2g)-0.5)*i = i*g/2 ; c = 2X + f*c
            ig = work_p.tile([128, 256], FP32, tag=f"ig{q}", name=f"ig{q}_{t}")
            nc.vector.scalar_tensor_tensor(
                ig[:, :], ifog[:, 768:1024], -0.5, ifog[:, 0:256],
                ALU.add, ALU.mult)
            if t == 0:
                c_new = state_p.tile([128, 256], FP32, tag=f"c{q}",
                                     name=f"c{q}_{t}")
                nc.vector.tensor_scalar_mul(c_new[:, :], ig[:, :], 2.0)
            else:
                c_new = state_p.tile([128, 256], FP32, tag=f"c{q}",
                                     name=f"c{q}_{t}")
                nc.vector.scalar_tensor_tensor(
                    c_new[:, :], ig[:, :], 2.0, fcs[:, :], ALU.mult, ALU.add)
            c_prev[q] = c_new
            th = work_p.tile([128, 256], FP32, tag=f"th{q}", name=f"th{q}_{t}")
            nc.scalar.activation(th[:, :], c_new[:, :], AF.Tanh)
            h_new = state_p.tile([128, 256], FP16, tag=f"h{q}",
                                 name=f"h{q}_{t}")
            nc.vector.tensor_tensor(h_new[:, :], ifog[:, 512:768], th[:, :],
                                    ALU.mult)
            h_prev[q] = h_new
            slot = tr * NB + q
            for j in range(2):
                nc.tensor.matmul(
                    fc_ps[:, slot * V:(slot + 1) * V],
                    h_new[:, j * 128:(j + 1) * 128],
                    wfc_sb[j][:, :],
                    start=(slot == 0 and j == 0),
                    stop=(slot == nslot - 1 and j == 1),
                )
            info[4] -= 1
            if info[4] == 0:
                emit_chunk_epilogue(ci)

        def emit_chunk_epilogue(ci):
            fc_ps, ohN, t0, nslot, _ = chunk_info.pop(ci)
            biased = work_p.tile([128, nslot * V], FP32, tag="biased",
                                 name=f"biased{ci}")
            nc.vector.tensor_tensor(
                biased[:, :], fc_ps[:, :], bfc_tile[:, :nslot * V], ALU.add)
            nc.scalar.activation(
                relu_ring[:, t0 * NB * V:(t0 * NB + nslot) * V],
                biased[:, :], AF.Relu)
            selp = work_p.tile([128, nslot * V], FP32, tag="selp",
                               name=f"selp{ci}")
            rr_ap = relu_ring[:, :]
            nc.vector.tensor_tensor(
                selp[:, :],
                AP(rr_ap.tensor, rr_ap.offset + t0 * NB * V,
                   [list(rr_ap.ap[0]), [1, nslot * V]]),
                ohN[:, :], ALU.mult)
            nc.vector.tensor_reduce(
                out=sel_ring[:, t0 * NB:t0 * NB + nslot],
                in_=AP(selp[:, :].tensor, selp[:, :].offset,
                       [list(selp[:, :].ap[0]), [V, nslot], [1, V]]),
                axis=mybir.AxisListType.X,
                op=ALU.add,
            )

        emit_chunk_prefetch(0)
        emit_oh(0, 0)
        emit_oh(1, 0)
        for t in range(nstep):
            ci = t // CH
            if t % CH == 0 and ci + 1 < n_chunks:
                emit_chunk_prefetch(ci + 1)
            emit_rec_act(0, t)
            emit_oh(0, t + 1)
            if t > 0:
                emit_dve(1, t - 1)
            emit_rec_act(1, t)
            emit_oh(1, t + 1)
            emit_dve(0, t)
        emit_dve(1, nstep - 1)

        # ---- end phase: exp/ln (one ACT table switch), log-probs, NLL ----
        logZ = ring_p.tile([128, nstep * NB], FP32, tag="logZ")
        n_slots_total = nstep * NB
        piece_slots = 64
        off = 0
        while off < n_slots_total:
            ps = min(piece_slots, n_slots_total - off)
            expx = end_p.tile([128, piece_slots * V], FP32, tag="expx")
            nc.scalar.activation(
                expx[:, :ps * V],
                relu_ring[:, off * V:(off + ps) * V], AF.Exp)
            nc.vector.tensor_reduce(
                out=logZ[:, off:off + ps],
                in_=AP(expx[:, :].tensor, expx[:, :].offset,
                       [list(expx[:, :].ap[0]), [V, ps], [1, V]]),
                axis=mybir.AxisListType.X,
                op=ALU.add,
            )
            off += ps
        sumsZ = logZ
        logZ = ring_p.tile([128, nstep * NB], FP32, tag="logZ2")
        nc.scalar.activation(logZ[:, :], sumsZ[:, :], AF.Ln)

        # log_probs = relu_logits - logZ (broadcast over V); DMA out per piece
        off = 0
        while off < n_slots_total:
            ps = min(piece_slots, n_slots_total - off)
            lp_sb = end_p.tile([128, piece_slots * V], FP32, tag="lp")
            nc.vector.tensor_tensor(
                lp_sb[:, :ps * V],
                AP(relu_ring[:, :].tensor, relu_ring[:, :].offset + off * V,
                   [list(relu_ring[:, :].ap[0]), [V, ps], [1, V]]),
                AP(logZ[:, :].tensor, logZ[:, :].offset + off,
                   [list(logZ[:, :].ap[0]), [1, ps], [0, V]]),
                ALU.subtract,
            )
            # slots are (t*NB + q); pieces are multiples of NB
            assert off % NB == 0 and (ps % NB == 0 or off + ps == n_slots_total)
            tcnt = ps // NB
            tstart = off // NB
            for q in range(NB):
                nc.sync.dma_start(
                    AP(lp_out.tensor, tstart * V + q * BC * nstep * V,
                       [[nstep * V, 128], [V, tcnt], [1, V]]),
                    AP(lp_sb[:, :].tensor, lp_sb[:, :].offset + q * V,
                       [list(lp_sb[:, :].ap[0]), [NB * V, tcnt], [1, V]]),
                )
            off += ps

        # mask[p, (t, q)] = (x[q*128+p, t+1] != IGNORE)
        mask = ring_p.tile([128, nstep * NB], FP32, tag="mask")
        nc.vector.tensor_scalar(
            mask[:, :],
            AP(xB[:, :, :].tensor, xB[:, :, :].offset + 1,
               [list(xB[:, :, :].ap[0]), [1, nstep], [T, NB]]),
            float(IGNORE), None, ALU.not_equal)

        # nll contribution per slot: (logZ - sel) * mask, then reduce over t
        nll_slot = ring_p.tile([128, nstep * NB], FP32, tag="nll_slot")
        nc.vector.tensor_tensor(
            nll_slot[:, :], logZ[:, :], sel_ring[:, :], ALU.subtract)
        nc.vector.tensor_tensor(
            nll_slot[:, :], nll_slot[:, :], mask[:, :], ALU.mult)
        nllt = end_p.tile([128, NB], FP32, tag="nllt")
        nc.vector.tensor_reduce(
            out=nllt[:, :],
            in_=AP(nll_slot[:, :].tensor, nll_slot[:, :].offset,
                   [list(nll_slot[:, :].ap[0]), [1, NB], [NB, nstep]]),
            axis=mybir.AxisListType.X,
            op=ALU.add,
        )
        nc.sync.dma_start(nll_out[:, :], nllt[:, :])

    nc.compile()
    return nc


_PROGRAM = None


def _get_program():
    global _PROGRAM
    if _PROGRAM is None:
        _PROGRAM = build_program(NCORES)
    return _PROGRAM


def host_prep(x, emb_table, w_ih, w_hh, b_ih, b_fc, b_hh, w_fc):
    """Fold weights and build the per-core input maps."""
    x = np.asarray(x).astype(np.int32)
    emb_table = np.asarray(emb_table, dtype=np.float32)
    W2 = emb_table @ np.asarray(w_ih, np.float32).T \
        + np.asarray(b_ih, np.float32) + np.asarray(b_hh, np.float32)
    W2[:, 512:768] *= 2.0
    W2 = W2.astype(np.float16)
    whhT = np.asarray(w_hh, np.float32).T.copy()
    whhT[:, 512:768] *= 2.0
    whhT = np.ascontiguousarray(whhT.astype(np.float16))
    wfcT = np.ascontiguousarray(np.asarray(w_fc, np.float32).T.astype(np.float16))
    bfc = np.asarray(b_fc, np.float32)

    in_maps = []
    for c in range(NCORES):
        xs = x[c * B:(c + 1) * B]  # [256, 128]
        xf = xs.reshape(-1)
        idx16 = np.zeros((16, NTOK // 16), np.int16)
        ii = np.arange(NTOK)
        idx16[ii % 16, ii // 16] = xf.astype(np.int16)
        idx16 = np.tile(idx16, (8, 1))
        in_maps.append({
            "xidx16": idx16,
            "xflat32": np.ascontiguousarray(xf.astype(np.float32)),
            "xT": np.ascontiguousarray(xs.T.astype(np.float32)),
            "whhT": whhT,
            "W2": np.ascontiguousarray(W2),
            "wfcT": wfcT,
            "bfc": bfc,
            "etab": emb_table,
        })
    return x, in_maps


def assemble(x, results):
    """Gather per-core outputs into the full-shape reference outputs."""
    lp = np.concatenate([r["lp_out"] for r in results], axis=0)
    emb = np.concatenate(
        [r["emb_out"].reshape(B, T, E) for r in results], axis=0)
    nll = np.stack([r["nll_out"] for r in results])  # [NC, 128, NB]
    # nll[c, p, q] is the masked NLL sum of batch element c*B + q*128 + p
    nll_sum = nll.transpose(0, 2, 1).reshape(B_FULL)
    lengths = (x != IGNORE).sum(axis=1).astype(np.float32)
    sample_loss = nll_sum / lengths
    mean_loss = np.float32(sample_loss.mean())
    return lp, emb, sample_loss, mean_loss


def kernel(x, emb_table, w_ih, w_hh, b_ih, b_fc, b_hh, w_fc):
    from concourse.bass_utils import run_bass_kernel_spmd

    x, in_maps = host_prep(x, emb_table, w_ih, w_hh, b_ih, b_fc, b_hh, w_fc)
    nc = _get_program()
    res = run_bass_kernel_spmd(nc, in_maps, core_ids=list(range(NCORES)))
    return assemble(x, res.results)
